# revision 21
# baseline (speedup 1.0000x reference)
"""Bidirectional MAMBA Trainium2 kernel, token-chunk software-pipelined.

Sharding (8 cores): (batch 2) x (direction 2) x (d_inner half 2).
Each core runs the full 4-layer chain of its direction on its batch with
E_loc=512 of the 1024 d_inner channels over the full N=1024 sequence
(backward stream is pre-flipped on the host).  Two pair-AllReduces per
layer chunk combine the u@Wx partials and the y@Wout partials.  The
selective scan runs exactly on the DVE via tensor_tensor_scan with S
state channels glued along the free dimension (dA zeroed at segment
starts resets the state), channels on partitions.

Pipelining: each layer is processed in CH=2 token chunks of Q=512.  The
DVE is the bottleneck engine (scan + elementwise ~230us/layer), so the
emission order threads everything else through the scan stream: while
the DVE scans chunk c, the tensor/scalar/CC engines run the Wout+AllReduce
+residual of chunk c-1 and the LN/Win/conv/Wx/AllReduce/dt prep of chunk
c+1, embedded at unit boundaries of the scan loop.  Scan state crosses
the chunk boundary via a per-unit carry column folded into dbu[.,si,0]
(h[0] = da[0]*h_in + dbu[0]) with da[.,si,0] zeroed afterwards.

Perf notes inherited from the unpipelined version: collectives in bf16;
LN stats broadcast via gpsimd partition_broadcast; activation-table
loads batched; scan-phase elementwise stays on the DVE (gpsimd Pool
measured ~3.1ns/elem and its SBUF traffic slows concurrent DVE scans
~2x).  The final direction-concat AllGather ships per chunk without the
flip (the combine reads backward-direction rows through reversed APs).

Self-contained: hardcodes all shapes; only needs trn_rl_repo on sys.path.
"""

import os
import sys

import numpy as np

for _p in ("/opt/trn_rl_repo", "/root/.axon_site/_ro/trn_rl_repo"):
    if os.path.isdir(_p) and _p not in sys.path:
        sys.path.insert(0, _p)

import ml_dtypes  # noqa: E402

import concourse.bacc as bacc  # noqa: E402
import concourse.bass as bass  # noqa: E402
import concourse.mybir as mybir  # noqa: E402
import concourse.tile as tile  # noqa: E402
from concourse import bass_utils, library_config  # noqa: E402

F32 = mybir.dt.float32
BF16 = mybir.dt.bfloat16
AF = mybir.ActivationFunctionType
OP = mybir.AluOpType

B, N, DM = 2, 1024, 512          # batch, seq, d_model
EFULL, S, RK, L, K = 1024, 16, 32, 4, 4  # d_inner, d_state, dt_rank, layers, conv
EL = EFULL // 2                  # d_inner per core (tensor-parallel half)
P = 128
KD = DM // P                     # 4 k-tiles over d_model
ET = EL // P                     # 4 tiles over local d_inner
CH = 2                           # token chunks per layer
Q = N // CH                      # tokens per chunk (512)
SG = 4                           # s-values glued per scan unit
NG = S // SG                     # 4 scan groups per chunk
GF = SG * Q                      # glued free size per scan unit (2048)
NU = NG * ET                     # 16 scan units per chunk
CQ = 256                         # combine token-chunk

_CACHE = {}


def _patch_act_tables():
    """Leave `natural_log_exp_and_others` as the only table providing Exp
    and Ln, so the act-table placement pass stops bouncing between the
    exp-only and ln-only sets (each reload costs ~1.3us of scalar time).
    Set ids are positional, so entries are edited in place, never
    reordered."""
    if _CACHE.get("actpatch"):
        return
    import functools
    import concourse.hw_specs as hw_specs
    import concourse.bacc as bacc_mod
    orig = hw_specs.get_activation_tables

    @functools.cache
    def patched(arch):
        tabs = {k: set(v) for k, v in orig(arch).items()}
        for name, fns in tabs.items():
            if name != "natural_log_exp_and_others":
                fns.discard(AF.Exp)
                fns.discard(AF.Ln)
        return tabs

    hw_specs.get_activation_tables = patched
    bacc_mod.get_activation_tables = patched
    _CACHE["actpatch"] = True


def _declare_inputs(nc):
    t = {}

    def d(name, shape, dt=F32):
        t[name] = nc.dram_tensor(name, list(shape), dt, kind="ExternalInput").ap()

    d("x_in", (DM, N))      # host passes x pre-transposed (F-layout)
    d("ident", (P, P))
    d("ones_col", (P, 1))
    d("ln_g", (L, DM)); d("ln_b", (L, DM))
    d("win", (L, DM, 2 * EL), BF16)      # cols = [u-half | z-half]
    d("convd", (L, K, ET, P, P), BF16)   # diagonalized depthwise conv weights
    d("conv_b", (L, EL))
    d("wx", (L, EL, RK + 2 * S), BF16)
    d("wdt", (L, RK, EL), BF16)
    d("bdt", (L, EL))
    d("amat", (L, EL, S))                # A = -exp(Alog) slice
    d("dvec", (L, EL))
    d("wout", (L, EL, DM), BF16)
    d("cg", (2 * DM,)); d("cb", (2 * DM,))
    d("cw", (2 * DM, DM), BF16)
    d("cbias", (DM,))
    out = nc.dram_tensor("out", [N, DM], F32, kind="ExternalOutput").ap()
    return t, out


def _build_program(sim_compat=False):
    key = ("prog", sim_compat)
    if key in _CACHE:
        return _CACHE[key]
    _patch_act_tables()
    nc = bacc.Bacc("TRN2", target_bir_lowering=False, debug=False, num_devices=8)
    t, out_ap = _declare_inputs(nc)
    with tile.TileContext(nc) as tc:
        _kernel(tc, t, out_ap, sim_compat)
    nc.compile()
    _CACHE[key] = nc
    return nc


def _kernel(tc, t, out_ap, sim_compat=False):
    nc = tc.nc
    import contextlib
    es = contextlib.ExitStack()

    eh_groups = [[0, 1], [2, 3], [4, 5], [6, 7]]       # d_inner-half pairs
    dir_groups = [[0, 2], [1, 3], [4, 6], [5, 7]]      # fwd/bwd pairs

    pers = es.enter_context(tc.tile_pool(name="pers", bufs=1))
    wp = es.enter_context(tc.tile_pool(name="wp", bufs=1))
    ck = es.enter_context(tc.tile_pool(name="ck", bufs=2))
    usb = es.enter_context(tc.tile_pool(name="usb", bufs=1))
    tp = es.enter_context(tc.tile_pool(name="tp", bufs=2))
    scn = es.enter_context(tc.tile_pool(name="scn", bufs=2))
    pm = es.enter_context(tc.tile_pool(name="pm", bufs=2, space="PSUM"))
    psY = es.enter_context(tc.tile_pool(name="psY", bufs=1, space="PSUM"))
    ps = es.enter_context(tc.tile_pool(name="ps", bufs=2, space="PSUM"))
    dram = es.enter_context(tc.tile_pool(name="dram", bufs=2, space="DRAM"))

    def apply_silu(out, psum_in, bias, uid):
        if not sim_compat:
            if bias is None:
                nc.scalar.activation(out, psum_in, AF.Silu)
            else:
                nc.scalar.activation(out, psum_in, AF.Silu, bias=bias)
            return
        tsg = tp.tile([P, Q], F32, tag="bcr", bufs=2, name=f"sg{uid}")
        tli = tp.tile([P, Q], F32, tag="bcr", bufs=2, name=f"sl{uid}")
        w = psum_in.shape[-1]
        if bias is None:
            nc.scalar.activation(tsg[:, 0:w], psum_in, AF.Sigmoid)
            nc.scalar.activation(tli[:, 0:w], psum_in, AF.Identity)
        else:
            nc.scalar.activation(tsg[:, 0:w], psum_in, AF.Sigmoid, bias=bias)
            nc.scalar.activation(tli[:, 0:w], psum_in, AF.Identity, bias=bias)
        nc.vector.tensor_mul(out, tli[:, 0:w], tsg[:, 0:w])

    # ---- persistent tiles
    x = [pers.tile([P, N], F32, tag=f"x{i}", name=f"x{i}") for i in range(KD)]
    ident_d = pers.tile([P, P], F32, tag="identd", name="ident_d")
    ident = pers.tile([P, P], F32, tag="ident", name="ident")
    ones_d = pers.tile([P, 1], F32, tag="onesd", name="ones_d")
    ones_col = pers.tile([P, 1], F32, tag="ones", name="ones_col")
    ones_bf = pers.tile([P, 1], BF16, tag="onesb", name="ones_bf")
    identb = pers.tile([P, P], BF16, tag="identb", name="identb")
    carry = pers.tile([P, NU * SG], BF16, tag="carry", name="carry")
    nc.sync.dma_start(ident_d[:], t["ident"])
    nc.vector.tensor_copy(ident[:], ident_d[:])
    nc.vector.tensor_copy(identb[:], ident_d[:])
    nc.sync.dma_start(ones_d[:], t["ones_col"])
    nc.vector.tensor_copy(ones_col[:], ones_d[:])
    nc.vector.tensor_copy(ones_bf[:], ones_d[:])

    # ---- load x directly in F-layout (host pre-transposed)
    for j in range(KD):
        nc.sync.dma_start(x[j][:], t["x_in"][j * P:(j + 1) * P, :])

    LS = {}   # per-layer state: weights, chunk tiles, dram tiles

    def load_weights(l):
        st = {}
        winw = [wp.tile([P, 2 * EL], BF16, tag=f"win{k}", name=f"win{l}_{k}")
                for k in range(KD)]
        for k in range(KD):
            nc.sync.dma_start(winw[k][:], t["win"][l, k * P:(k + 1) * P, :])
        convw = [[wp.tile([P, P], BF16, tag=f"cv{j}_{m}", name=f"cv{l}_{j}_{m}")
                  for m in range(ET)] for j in range(K)]
        for j in range(K):
            for m in range(ET):
                nc.sync.dma_start(convw[j][m][:], t["convd"][l, j, m])
        wxw = [wp.tile([P, RK + 2 * S], BF16, tag=f"wx{k}", name=f"wx{l}_{k}")
               for k in range(ET)]
        for k in range(ET):
            nc.sync.dma_start(wxw[k][:], t["wx"][l, k * P:(k + 1) * P, :])
        wdtw = wp.tile([RK, EL], BF16, tag="wdt", name=f"wdt{l}")
        nc.sync.dma_start(wdtw[:], t["wdt"][l])
        woutw = [wp.tile([P, DM], BF16, tag=f"wo{k}", name=f"wo{l}_{k}")
                 for k in range(ET)]
        for k in range(ET):
            nc.sync.dma_start(woutw[k][:], t["wout"][l, k * P:(k + 1) * P, :])
        amat = [wp.tile([P, S], F32, tag=f"am{m}", name=f"am{l}_{m}")
                for m in range(ET)]
        for m in range(ET):
            nc.sync.dma_start(amat[m][:], t["amat"][l, m * P:(m + 1) * P, :])
        pcol = [[wp.tile([P, 1], F32, tag=f"pc{w}_{m}", name=f"pc{w}{l}_{m}")
                 for m in range(ET)] for w in range(3)]
        for m in range(ET):
            sl = slice(m * P, (m + 1) * P)
            nc.sync.dma_start(pcol[0][m][:], t["bdt"][l, sl].unsqueeze(-1))
            nc.sync.dma_start(pcol[1][m][:], t["conv_b"][l, sl].unsqueeze(-1))
            nc.sync.dma_start(pcol[2][m][:], t["dvec"][l, sl].unsqueeze(-1))
        gcol = [wp.tile([P, 1], F32, tag=f"gc{i}", name=f"gc{l}_{i}")
                for i in range(KD)]
        bcol = [wp.tile([P, 1], F32, tag=f"bc{i}", name=f"bc{l}_{i}")
                for i in range(KD)]
        for i in range(KD):
            sl = slice(i * P, (i + 1) * P)
            nc.sync.dma_start(gcol[i][:], t["ln_g"][l, sl].unsqueeze(-1))
            nc.sync.dma_start(bcol[i][:], t["ln_b"][l, sl].unsqueeze(-1))
        st.update(winw=winw, convw=convw, wxw=wxw, wdtw=wdtw, woutw=woutw,
                  amat=amat, bdtc=pcol[0], cbc=pcol[1], dvc=pcol[2],
                  gcol=gcol, bcol=bcol, usb=[None] * ET, ch={})
        LS[l] = st

    def emit_A(l, c):
        """LN -> Win-u -> conv -> silu -> Wx -> AllReduce -> z -> dt -> vb
        for chunk c of layer l."""
        st = LS[l]
        cs = slice(c * Q, (c + 1) * Q)
        uid = f"{l}_{c}"
        # -- LN stats over the feature (partition) axis
        sxq = ps.tile([33, Q], F32, tag="st", name=f"sxq{uid}")
        sx, sq = sxq[0:1], sxq[32:33]
        for i in range(KD):
            xsq = tp.tile([P, Q], BF16, tag="xsqb", bufs=2, name=f"xsq{uid}_{i}")
            nc.scalar.square(xsq[:], x[i][:, cs])
            nc.tensor.matmul(sx[:], ones_col[:], x[i][:, cs],
                             start=(i == 0), stop=(i == KD - 1))
            nc.tensor.matmul(sq[:], ones_bf[:], xsq[:],
                             start=(i == 0), stop=(i == KD - 1))
        nm = ck.tile([1, Q], BF16, tag="nm", name=f"nm{uid}")
        rstd = ck.tile([1, Q], BF16, tag="rstd", name=f"rstd{uid}")
        nc.vector.tensor_scalar_mul(nm[0:1], sx[:], -1.0 / DM)
        nc.vector.tensor_scalar_mul(rstd[0:1], sq[:], 1.0 / DM)
        nc.vector.tensor_mul(sq[:], nm[0:1], nm[0:1])
        nc.vector.tensor_sub(rstd[0:1], rstd[0:1], sq[:])
        nc.vector.tensor_scalar_add(rstd[0:1], rstd[0:1], 1e-5)
        nc.scalar.activation(rstd[0:1], rstd[0:1], AF.Ln)
        nc.scalar.activation(rstd[0:1], rstd[0:1], AF.Exp, scale=-0.5)
        nmb = ck.tile([P, Q], BF16, tag="nmb", name=f"nmb{uid}")
        rsb = ck.tile([P, Q], BF16, tag="rsb", name=f"rsb{uid}")
        nc.gpsimd.partition_broadcast(nmb[:], nm[0:1])
        nc.gpsimd.partition_broadcast(rsb[:], rstd[0:1])
        hln = [ck.tile([P, Q], BF16, tag=f"hln{i}", name=f"hln{uid}_{i}")
               for i in range(KD)]
        for i in range(KD):
            t1 = tp.tile([P, Q], F32, tag="t4", bufs=2, name=f"lnt1_{uid}_{i}")
            nc.vector.tensor_add(t1[:], x[i][:, cs], nmb[:])
            nc.vector.tensor_mul(t1[:], t1[:], rsb[:])
            nc.vector.tensor_scalar(hln[i][:], t1[:], st["gcol"][i][:],
                                    st["bcol"][i][:], op0=OP.mult, op1=OP.add)
        # -- Win u-wave + conv + silu
        usi = [ck.tile([P, Q], BF16, tag=f"usi{m}", name=f"usi{uid}_{m}")
               for m in range(ET)]
        for m in range(ET):
            pu = pm.tile([P, Q], F32, tag="gemm", name=f"pu{uid}_{m}")
            for k in range(KD):
                nc.tensor.matmul(pu[:], st["winw"][k][:, m * P:(m + 1) * P],
                                 hln[k][:], start=(k == 0), stop=(k == KD - 1))
            if c == 0:
                st["usb"][m] = usb.tile([P, K - 1 + N], BF16, tag=f"usb{m}",
                                        name=f"usb{l}_{m}")
                nc.vector.memset(st["usb"][m][:, 0:K - 1], 0.0)
            u_sb = st["usb"][m]
            nc.scalar.copy(u_sb[:, K - 1 + c * Q:K - 1 + (c + 1) * Q], pu[:])
            pc = pm.tile([P, Q], F32, tag="gemm", name=f"pcv{uid}_{m}")
            for j in range(K):
                nc.tensor.matmul(pc[:], st["convw"][j][m][:],
                                 u_sb[:, c * Q + j:c * Q + j + Q],
                                 start=(j == 0), stop=(j == K - 1))
            apply_silu(usi[m][:], pc[:], st["cbc"][m][:], f"u{uid}_{m}")
        # -- Wx partial GEMM + pair AllReduce (bf16)
        px = pm.tile([P, Q], F32, tag="gemm", name=f"px{uid}")
        for k in range(ET):
            nc.tensor.matmul(px[0:RK + 2 * S, :], st["wxw"][k][:], usi[k][:],
                             start=(k == 0), stop=(k == ET - 1))
        xdp = dram.tile([RK + 2 * S, Q], BF16, tag="xdp", name=f"xdp{uid}")
        xds = dram.tile([RK + 2 * S, Q], BF16, tag="xds", name=f"xds{uid}")
        pxs = tp.tile([RK + 2 * S, Q], BF16, tag="pxsb", bufs=2,
                      name=f"pxs{uid}")
        nc.scalar.copy(pxs[:], px[0:RK + 2 * S, :])
        nc.sync.dma_start(xdp[:], pxs[:])
        nc.gpsimd.collective_compute(
            "AllReduce", OP.add, replica_groups=eh_groups,
            ins=[xdp[:]], outs=[xds[:]])
        # -- Win z-wave + silu (overlaps the collective)
        zsi = [ck.tile([P, Q], BF16, tag=f"zsi{m}", name=f"zsi{uid}_{m}")
               for m in range(ET)]
        for m in range(ET):
            pz = pm.tile([P, Q], F32, tag="gemm", name=f"pz{uid}_{m}")
            for k in range(KD):
                nc.tensor.matmul(pz[:],
                                 st["winw"][k][:, (ET + m) * P:(ET + m + 1) * P],
                                 hln[k][:], start=(k == 0), stop=(k == KD - 1))
            apply_silu(zsi[m][:], pz[:], None, f"z{uid}_{m}")
        st["ch"][c] = dict(usi=usi, zsi=zsi, xds=xds)

    def emit_A_dt(l, c):
        """dt = softplus(xdbl[:,:RK] @ Wdt + bdt).  Emitted well after the
        Wx AllReduce was issued, so the scalar queue is not convoyed behind
        the collective round trip."""
        st = LS[l]
        chs = st["ch"][c]
        uid = f"{l}_{c}"
        xdbl_bf = ck.tile([RK, Q], BF16, tag="xdblb", name=f"xdblb{uid}")
        nc.gpsimd.dma_start(xdbl_bf[:], chs["xds"][0:RK, :])
        dtb = [ck.tile([P, Q], BF16, tag=f"dtb{m}", name=f"dtb{uid}_{m}")
               for m in range(ET)]
        spxs = []
        for m in range(ET):
            pd = pm.tile([P, Q], F32, tag="gemm", name=f"pd{uid}_{m}")
            nc.tensor.matmul(pd[:], st["wdtw"][:, m * P:(m + 1) * P],
                             xdbl_bf[:], start=True, stop=True)
            spx = tp.tile([P, Q], BF16, tag="spx", bufs=4, name=f"spx{uid}_{m}")
            nc.scalar.activation(spx[:], pd[:], AF.Exp, bias=st["bdtc"][m][:])
            spxs.append(spx)
        for m in range(ET):
            nc.scalar.activation(dtb[m][:], spxs[m][:], AF.Ln,
                                 bias=ones_col[:])
        chs["dtb"] = dtb

    def emit_B(l, c, embeds):
        """Scan units for chunk c of layer l; embeds[u] callables are
        emitted after unit u (to thread other chunks' work through the
        DVE-paced stream)."""
        st = LS[l]
        chs = st["ch"][c]
        vb = [ck.tile([P, Q], BF16, tag=f"vb{m}", name=f"vb{l}_{c}_{m}")
              for m in range(ET)]
        for m in range(ET):
            nc.vector.tensor_mul(vb[m][:], chs["dtb"][m][:],
                                 chs["usi"][m][:])
        chs["vb"] = vb
        yacc = [psY.tile([P, Q], F32, tag=f"yac{m}", name=f"yac{l}_{c}_{m}")
                for m in range(ET)]
        chs["yacc"] = yacc
        for g in range(NG):
            bb = scn.tile([P, GF], BF16, tag="bb", name=f"bb{l}_{c}_{g}")
            cc = scn.tile([P, GF], BF16, tag="cc", name=f"cc{l}_{c}_{g}")
            nc.sync.dma_start(
                bb[:],
                chs["xds"][RK + g * SG:RK + (g + 1) * SG,
                           :].partition_broadcast(P))
            nc.sync.dma_start(
                cc[:],
                chs["xds"][RK + S + g * SG:RK + S + (g + 1) * SG,
                           :].partition_broadcast(P))
            bb3 = bb.rearrange("p (s n) -> p s n", s=SG)
            for m in range(ET):
                u = g * ET + m
                uid = f"{l}_{c}_{g}_{m}"
                da = scn.tile([P, GF], BF16, tag="da", name=f"da{uid}")
                for si in range(SG):
                    s = g * SG + si
                    nc.scalar.activation(da[:, si * Q:(si + 1) * Q],
                                         chs["dtb"][m][:], AF.Exp,
                                         scale=st["amat"][m][:, s:s + 1])
                da3 = da.rearrange("p (s n) -> p s n", s=SG)
                dbu = scn.tile([P, GF], BF16, tag="dbu", bufs=2,
                               name=f"dbu{uid}")
                vv = chs["vb"][m].unsqueeze(1).broadcast_to((P, SG, Q))
                db3 = dbu.rearrange("p (s n) -> p s n", s=SG)
                nc.vector.tensor_mul(db3[:], vv, bb3[:])
                if c > 0:
                    # fold the cross-chunk carry into dbu[., si, 0]
                    for si in range(SG):
                        col = u * SG + si
                        nc.vector.scalar_tensor_tensor(
                            db3[:, si, 0:1], da3[:, si, 0:1],
                            carry[:, col:col + 1], db3[:, si, 0:1],
                            op0=OP.mult, op1=OP.add)
                nc.vector.memset(da3[:, :, 0:1], 0.0)
                hh = scn.tile([P, GF], BF16, tag="dbu", bufs=2,
                              name=f"hh{uid}")
                nc.vector.tensor_tensor_scan(hh[:], da[:], dbu[:], 0.0,
                                             op0=OP.mult, op1=OP.add)
                if c < CH - 1:
                    hh3 = hh.rearrange("p (s n) -> p s n", s=SG)
                    nc.vector.tensor_copy(
                        carry[:, u * SG:(u + 1) * SG], hh3[:, :, Q - 1])
                ym = scn.tile([P, GF], BF16, tag="ymt", bufs=2,
                              name=f"ym{uid}")
                nc.vector.tensor_mul(ym[:], hh[:], cc[:])
                for si in range(SG):
                    nc.tensor.matmul(yacc[m][:], identb[:],
                                     ym[:, si * Q:(si + 1) * Q],
                                     start=(g == 0 and si == 0),
                                     stop=(g == NG - 1 and si == SG - 1))
                if g == NG - 1:
                    # gate m as soon as its yacc closes; Wout k-major so
                    # its first matmuls overlap the remaining scan units
                    gated = chs.setdefault("gated", [None] * ET)
                    gated[m] = ck.tile([P, Q], BF16, tag=f"gt{m}",
                                       name=f"gt{l}_{c}_{m}")
                    nc.vector.scalar_tensor_tensor(
                        gated[m][:], chs["usi"][m][:], st["dvc"][m][:],
                        yacc[m][:], op0=OP.mult, op1=OP.add)
                    nc.vector.tensor_mul(gated[m][:], gated[m][:],
                                         chs["zsi"][m][:])
                for fn in embeds.get(u, ()):
                    fn()

    def emit_C(l, c):
        """Wout partial + pair AllReduce for chunk c (residual deferred)."""
        st = LS[l]
        chs = st["ch"][c]
        uid = f"{l}_{c}"
        dxp = dram.tile([DM, Q], BF16, tag="dxp", name=f"dxp{uid}")
        dxs = dram.tile([DM, Q], BF16, tag="dxs", name=f"dxs{uid}")
        chs["dxs"] = dxs
        po = [psY.tile([P, Q], F32, tag=f"yac{mo}", name=f"po{uid}_{mo}")
              for mo in range(KD)]
        for k in range(ET):
            for mo in range(KD):
                nc.tensor.matmul(po[mo][:],
                                 st["woutw"][k][:, mo * P:(mo + 1) * P],
                                 chs["gated"][k][:],
                                 start=(k == 0), stop=(k == ET - 1))
        for mo in range(KD):
            pos = tp.tile([P, Q], BF16, tag="bpd", bufs=2,
                          name=f"pos{uid}_{mo}")
            nc.scalar.copy(pos[:], po[mo][:])
            nc.sync.dma_start(dxp[mo * P:(mo + 1) * P, :], pos[:])
            if mo % 2 == 1:
                nc.gpsimd.collective_compute(
                    "AllReduce", OP.add, replica_groups=eh_groups,
                    ins=[dxp[(mo - 1) * P:(mo + 1) * P, :]],
                    outs=[dxs[(mo - 1) * P:(mo + 1) * P, :]])

    def emit_resid(l, c, mos):
        st = LS[l]
        chs = st["ch"][c]
        cs = slice(c * Q, (c + 1) * Q)
        for mo in mos:
            dxt = tp.tile([P, Q], BF16, tag="bpd", bufs=2,
                          name=f"dxt{l}_{c}_{mo}")
            nc.gpsimd.dma_start(dxt[:], chs["dxs"][mo * P:(mo + 1) * P, :])
            nc.vector.tensor_add(x[mo][:, cs], x[mo][:, cs], dxt[:])

    # ================= combine helpers =================
    DM2 = DM + 2
    cat_part = [dram.tile([DM2, Q], BF16, tag=f"catp{c}", bufs=1,
                          name=f"cat_part{c}") for c in range(CH)]
    cat_sum = [dram.tile([2 * DM2, Q], BF16, tag=f"cats{c}", bufs=1,
                         name=f"cat_sum{c}") for c in range(CH)]

    def emit_cat(c):
        """Ship our direction's chunk-c output (straight, no flip) plus its
        LN partial stats, then AllGather the dir pair."""
        cs = slice(c * Q, (c + 1) * Q)
        sxq = ps.tile([33, Q], F32, tag="st", name=f"csxq{c}")
        for i in range(KD):
            sf = tp.tile([P, Q], BF16, tag="bpd", bufs=2, name=f"sf{c}_{i}")
            nc.vector.tensor_copy(sf[:], x[i][:, cs])
            nc.sync.dma_start(cat_part[c][i * P:(i + 1) * P, :], sf[:])
            xsq = tp.tile([P, Q], BF16, tag="xsqb", bufs=2, name=f"pxq{c}_{i}")
            nc.scalar.square(xsq[:], sf[:])
            nc.tensor.matmul(sxq[0:1, :], ones_bf[:], sf[:],
                             start=(i == 0), stop=(i == KD - 1))
            nc.tensor.matmul(sxq[32:33, :], ones_bf[:], xsq[:],
                             start=(i == 0), stop=(i == KD - 1))
        stats_sb = tp.tile([33, Q], BF16, tag="cstat", bufs=2,
                           name=f"stats_sb{c}")
        nc.scalar.copy(stats_sb[0:1, :], sxq[0:1, :])
        nc.scalar.copy(stats_sb[32:33, :], sxq[32:33, :])
        nc.sync.dma_start(cat_part[c][DM:DM + 1, :], stats_sb[0:1, :])
        nc.sync.dma_start(cat_part[c][DM + 1:DM + 2, :], stats_sb[32:33, :])
        nc.gpsimd.collective_compute(
            "AllGather", OP.bypass, replica_groups=dir_groups,
            ins=[cat_part[c][:]], outs=[cat_sum[c][:]])

    # ================= emission schedule =================
    load_weights(0)
    # combine weights prefetch (idle DMA time at the start)
    cww = [wp.tile([P, DM], BF16, tag=f"cwt{k}", name=f"cw{k}")
           for k in range(2 * KD)]
    for k in range(2 * KD):
        nc.sync.dma_start(cww[k][:], t["cw"][k * P:(k + 1) * P, :])
    cbias_c = [wp.tile([P, 1], F32, tag=f"cbs{m}", name=f"cbs{m}")
               for m in range(KD)]
    for m in range(KD):
        nc.sync.dma_start(cbias_c[m][:],
                          t["cbias"][m * P:(m + 1) * P].unsqueeze(-1))
    cgcol = [wp.tile([P, 1], F32, tag="cgcol", bufs=8, name=f"cgc{i}")
             for i in range(2 * KD)]
    cbcol = [wp.tile([P, 1], F32, tag="cbcol", bufs=8, name=f"cbc{i}")
             for i in range(2 * KD)]
    for i in range(2 * KD):
        nc.sync.dma_start(cgcol[i][:], t["cg"][i * P:(i + 1) * P].unsqueeze(-1))
        nc.sync.dma_start(cbcol[i][:], t["cb"][i * P:(i + 1) * P].unsqueeze(-1))
    emit_A(0, 0)
    emit_A_dt(0, 0)
    for l in range(L):
        embeds0 = {}
        if l > 0:
            embeds0[2] = [lambda l=l: emit_resid(l - 1, 1, (0, 1))]
            embeds0[3] = [lambda l=l: emit_resid(l - 1, 1, (2, 3))]
        embeds0[4] = [lambda l=l: emit_A(l, 1)]
        embeds0[9] = [lambda l=l: emit_A_dt(l, 1)]
        emit_B(l, 0, embeds0)
        emit_C(l, 0)
        embeds1 = {
            2: [lambda l=l: emit_resid(l, 0, (0, 1))],
            3: [lambda l=l: emit_resid(l, 0, (2, 3))],
        }
        if l < L - 1:
            embeds1[4] = [lambda l=l: (load_weights(l + 1),
                                       emit_A(l + 1, 0))]
            embeds1[9] = [lambda l=l: emit_A_dt(l + 1, 0)]
        else:
            embeds1[5] = [lambda: emit_cat(0)]
        emit_B(l, 1, embeds1)
        emit_C(l, 1)
    emit_resid(L - 1, 1, (0, 1, 2, 3))
    emit_cat(1)

    # ================= combine =================
    def cat_row(i, c):
        """Feature-tile i of chunk c of the concat layout."""
        if i < KD:
            return cat_sum[c][i * P:(i + 1) * P, :]
        return cat_sum[c][DM2 + (i - KD) * P:DM2 + (i - KD + 1) * P, :]

    # global LN stats in OUTPUT token order: fwd stats straight + bwd
    # stats column-reversed (bwd stream position p holds token N-1-p).
    cnm = ck.tile([1, N], BF16, tag="cnm", name="cnm")
    crstd = ck.tile([1, N], BF16, tag="crstd", name="crstd")
    sxb = ck.tile([1, N], BF16, tag="sxb", name="sxb")
    sqb = ck.tile([1, N], BF16, tag="sqb", name="sqb")
    for c in range(CH):
        cs = slice(c * Q, (c + 1) * Q)
        nc.sync.dma_start(cnm[0:1, cs], cat_sum[c][DM:DM + 1, :])
        nc.sync.dma_start(crstd[0:1, cs], cat_sum[c][DM + 1:DM + 2, :])
        nc.sync.dma_start(sxb[0:1, cs], cat_sum[c][DM2 + DM:DM2 + DM + 1, :])
        nc.sync.dma_start(sqb[0:1, cs],
                          cat_sum[c][DM2 + DM + 1:DM2 + DM + 2, :])
    nc.vector.tensor_add(cnm[0:1, :], cnm[0:1, :], sxb[0:1, ::-1])
    nc.vector.tensor_add(crstd[0:1, :], crstd[0:1, :], sqb[0:1, ::-1])
    nc.vector.tensor_scalar_mul(cnm[0:1, :], cnm[0:1, :], -1.0 / (2 * DM))
    nc.vector.tensor_scalar_mul(crstd[0:1, :], crstd[0:1, :], 1.0 / (2 * DM))
    nc.vector.tensor_mul(sxb[0:1, :], cnm[0:1, :], cnm[0:1, :])
    nc.vector.tensor_sub(crstd[0:1, :], crstd[0:1, :], sxb[0:1, :])
    nc.vector.tensor_scalar_add(crstd[0:1, :], crstd[0:1, :], 1e-5)
    nc.scalar.activation(crstd[0:1, :], crstd[0:1, :], AF.Ln)
    nc.scalar.activation(crstd[0:1, :], crstd[0:1, :], AF.Exp, scale=-0.5)
    cnmb = ck.tile([P, N], BF16, tag="cnmb", bufs=1, name="cnmb")
    crsb = ck.tile([P, N], BF16, tag="crsb", bufs=1, name="crsb")
    nc.gpsimd.partition_broadcast(cnmb[:], cnm[0:1, :])
    nc.gpsimd.partition_broadcast(crsb[:], crstd[0:1, :])

    for q in range(N // CQ):
        qs = slice(q * CQ, (q + 1) * CQ)
        rq = N - (q + 1) * CQ      # bwd source cols (to be read reversed)
        xc = [ck.tile([P, CQ], BF16, tag="xc", bufs=9, name=f"xc{q}_{i}")
              for i in range(2 * KD)]
        for i in range(2 * KD):
            if i < KD:
                c0, o0 = divmod(q * CQ, Q)
                src = cat_row(i, c0)[:, o0:o0 + CQ]
            else:
                c0, o0 = divmod(rq, Q)
                src = cat_row(i, c0)[:, o0:o0 + CQ]
            nc.sync.dma_start(xc[i][:], src)
        hcq = [ck.tile([P, CQ], BF16, tag="hc", bufs=9, name=f"hc{q}_{i}")
               for i in range(2 * KD)]
        for i in range(2 * KD):
            xin = xc[i][:, :] if i < KD else xc[i][:, ::-1]
            t1c = tp.tile([P, CQ], F32, tag="lnt1c", bufs=2, name=f"t1c{q}_{i}")
            nc.vector.tensor_add(t1c[:], xin, cnmb[:, qs])
            nc.vector.tensor_mul(t1c[:], t1c[:], crsb[:, qs])
            nc.vector.tensor_scalar(hcq[i][:], t1c[:], cgcol[i][:],
                                    cbcol[i][:], op0=OP.mult, op1=OP.add)
        ot = tp.tile([P, DM], F32, tag="tio", bufs=2, name=f"ot{q}_a")
        ot2 = tp.tile([P, DM], F32, tag="tio", bufs=2, name=f"ot{q}_b")
        for m in range(KD):
            pg = pm.tile([P, Q], F32, tag="gemm", name=f"pg{q}_{m}")
            for k in range(2 * KD):
                nc.tensor.matmul(pg[:, 0:CQ], cww[k][:, m * P:(m + 1) * P],
                                 hcq[k][:], start=(k == 0),
                                 stop=(k == 2 * KD - 1))
            ogm = tp.tile([P, CQ], F32, tag="og", bufs=2, name=f"og{q}_{m}")
            gfn = AF.Identity if sim_compat else AF.Gelu
            nc.scalar.activation(ogm[:, 0:CQ], pg[:, 0:CQ], gfn,
                                 bias=cbias_c[m][:])
            for hh2 in range(CQ // P):
                pts = pm.tile([P, Q], F32, tag="gemm", name=f"otp{q}_{m}_{hh2}")
                nc.tensor.transpose(
                    pts[:, 0:P], ogm[:, hh2 * P:(hh2 + 1) * P], ident[:])
                dst = ot if hh2 == 0 else ot2
                nc.scalar.copy(dst[:, m * P:(m + 1) * P], pts[:, 0:P])
        nc.sync.dma_start(out_ap[q * CQ:q * CQ + P, :], ot[:])
        nc.sync.dma_start(out_ap[q * CQ + P:(q + 1) * CQ, :], ot2[:])

    es.close()


# ----------------------------------------------------------------- host side
def _bf(a):
    return np.asarray(a, dtype=np.float32).astype(ml_dtypes.bfloat16)


def _core_inputs(inputs, b, dirn, e):
    pre = "fwd" if dirn == 0 else "bwd"
    g = lambda n: np.asarray(inputs[pre + "_" + n], dtype=np.float32)
    x = np.asarray(inputs["x"], dtype=np.float32)[b]          # (N, DM)
    if dirn == 1:
        x = x[::-1]
    es = slice(e * EL, (e + 1) * EL)

    win_full = g("Win")                                        # (L, DM, 2*EFULL)
    win = np.concatenate(
        [win_full[:, :, e * EL:(e + 1) * EL],
         win_full[:, :, EFULL + e * EL:EFULL + (e + 1) * EL]], axis=2)

    cw4 = g("conv_w")[:, es, 0, :]                             # (L, EL, K)
    convd = np.zeros((L, K, ET, P, P), np.float32)
    for j in range(K):
        for m in range(ET):
            for l in range(L):
                np.fill_diagonal(convd[l, j, m], cw4[l, m * P:(m + 1) * P, j])

    return {
        "x_in": np.ascontiguousarray(x.T),
        "ident": np.eye(P, dtype=np.float32),
        "ones_col": np.ones((P, 1), np.float32),
        "ln_g": g("ln_g"), "ln_b": g("ln_b"),
        "win": _bf(win),
        "convd": _bf(convd),
        "conv_b": g("conv_b")[:, es],
        "wx": _bf(g("Wx")[:, es, :]),
        "wdt": _bf(g("Wdt")[:, :, es]),
        "bdt": g("bdt")[:, es],
        "amat": -np.exp(g("Alog")[:, es, :]),
        "dvec": g("D")[:, es],
        "wout": _bf(g("Wout")[:, es, :]),
        "cg": np.asarray(inputs["cmb_ln_g"], np.float32),
        "cb": np.asarray(inputs["cmb_ln_b"], np.float32),
        "cw": _bf(np.asarray(inputs["cmb_W"], np.float32)),
        "cbias": np.asarray(inputs["cmb_b"], np.float32),
    }


def make_in_maps(inputs):
    in_maps = []
    for b in range(B):
        for dirn in range(2):
            for e in range(2):
                in_maps.append(_core_inputs(inputs, b, dirn, e))
    return in_maps


def kernel(**inputs):
    nc = _build_program()
    res = bass_utils.run_bass_kernel_spmd(nc, make_in_maps(inputs),
                                          list(range(8)))
    out = np.empty((B, N, DM), np.float32)
    for b in range(B):
        out[b] = res.results[b * 4]["out"]
    return out


if __name__ == "__main__":
    nc = _build_program()
    n_inst = sum(len(bb.instructions) for f in nc.m.functions for bb in f.blocks)
    print("program built ok:", n_inst, "instructions")


# revision 22
# speedup vs baseline: 1.0177x; 1.0177x over previous
"""Bidirectional MAMBA Trainium2 kernel, token-chunk software-pipelined.

Sharding (8 cores): (batch 2) x (direction 2) x (d_inner half 2).
Each core runs the full 4-layer chain of its direction on its batch with
E_loc=512 of the 1024 d_inner channels over the full N=1024 sequence
(backward stream is pre-flipped on the host).  Two pair-AllReduces per
layer chunk combine the u@Wx partials and the y@Wout partials.  The
selective scan runs exactly on the DVE via tensor_tensor_scan with S
state channels glued along the free dimension (dA zeroed at segment
starts resets the state), channels on partitions.

Pipelining: each layer is processed in CH=2 token chunks of Q=512.  The
DVE is the bottleneck engine (scan + elementwise ~230us/layer), so the
emission order threads everything else through the scan stream: while
the DVE scans chunk c, the tensor/scalar/CC engines run the Wout+AllReduce
+residual of chunk c-1 and the LN/Win/conv/Wx/AllReduce/dt prep of chunk
c+1, embedded at unit boundaries of the scan loop.  Scan state crosses
the chunk boundary via a per-unit carry column folded into dbu[.,si,0]
(h[0] = da[0]*h_in + dbu[0]) with da[.,si,0] zeroed afterwards.

Perf notes inherited from the unpipelined version: collectives in bf16;
LN stats broadcast via gpsimd partition_broadcast; activation-table
loads batched; scan-phase elementwise stays on the DVE (gpsimd Pool
measured ~3.1ns/elem and its SBUF traffic slows concurrent DVE scans
~2x).  The final direction-concat AllGather ships per chunk without the
flip (the combine reads backward-direction rows through reversed APs).

Self-contained: hardcodes all shapes; only needs trn_rl_repo on sys.path.
"""

import os
import sys

import numpy as np

for _p in ("/opt/trn_rl_repo", "/root/.axon_site/_ro/trn_rl_repo"):
    if os.path.isdir(_p) and _p not in sys.path:
        sys.path.insert(0, _p)

import ml_dtypes  # noqa: E402

import concourse.bacc as bacc  # noqa: E402
import concourse.bass as bass  # noqa: E402
import concourse.mybir as mybir  # noqa: E402
import concourse.tile as tile  # noqa: E402
from concourse import bass_utils, library_config  # noqa: E402

F32 = mybir.dt.float32
BF16 = mybir.dt.bfloat16
AF = mybir.ActivationFunctionType
OP = mybir.AluOpType

B, N, DM = 2, 1024, 512          # batch, seq, d_model
EFULL, S, RK, L, K = 1024, 16, 32, 4, 4  # d_inner, d_state, dt_rank, layers, conv
EL = EFULL // 2                  # d_inner per core (tensor-parallel half)
P = 128
KD = DM // P                     # 4 k-tiles over d_model
ET = EL // P                     # 4 tiles over local d_inner
CH = 2                           # token chunks per layer
Q = N // CH                      # tokens per chunk (512)
SG = 4                           # s-values glued per scan unit
NG = S // SG                     # 4 scan groups per chunk
GF = SG * Q                      # glued free size per scan unit (2048)
NU = NG * ET                     # 16 scan units per chunk
CQ = 256                         # combine token-chunk

_CACHE = {}


def _patch_act_tables():
    """Leave `natural_log_exp_and_others` as the only table providing Exp
    and Ln, so the act-table placement pass stops bouncing between the
    exp-only and ln-only sets (each reload costs ~1.3us of scalar time).
    Set ids are positional, so entries are edited in place, never
    reordered."""
    if _CACHE.get("actpatch"):
        return
    import functools
    import concourse.hw_specs as hw_specs
    import concourse.bacc as bacc_mod
    orig = hw_specs.get_activation_tables

    @functools.cache
    def patched(arch):
        tabs = {k: set(v) for k, v in orig(arch).items()}
        for name, fns in tabs.items():
            if name != "natural_log_exp_and_others":
                fns.discard(AF.Exp)
                fns.discard(AF.Ln)
        return tabs

    hw_specs.get_activation_tables = patched
    bacc_mod.get_activation_tables = patched
    _CACHE["actpatch"] = True


def _declare_inputs(nc):
    t = {}

    def d(name, shape, dt=F32):
        t[name] = nc.dram_tensor(name, list(shape), dt, kind="ExternalInput").ap()

    d("x_in", (DM, N))      # host passes x pre-transposed (F-layout)
    d("ident", (P, P))
    d("ones_col", (P, 1))
    d("ln_g", (L, DM)); d("ln_b", (L, DM))
    d("win", (L, DM, 2 * EL), BF16)      # cols = [u-half | z-half]
    d("convd", (L, K, ET, P, P), BF16)   # diagonalized depthwise conv weights
    d("conv_b", (L, EL))
    d("wx", (L, EL, RK + 2 * S), BF16)
    d("wdt", (L, RK, EL), BF16)
    d("bdt", (L, EL))
    d("amat", (L, EL, S))                # A = -exp(Alog) slice
    d("dvec", (L, EL))
    d("wout", (L, EL, DM), BF16)
    d("cg", (2 * DM,)); d("cb", (2 * DM,))
    d("cw", (2 * DM, DM), BF16)
    d("cbias", (DM,))
    out = nc.dram_tensor("out", [N, DM], F32, kind="ExternalOutput").ap()
    return t, out


def _build_program(sim_compat=False):
    key = ("prog", sim_compat)
    if key in _CACHE:
        return _CACHE[key]
    _patch_act_tables()
    nc = bacc.Bacc("TRN2", target_bir_lowering=False, debug=False, num_devices=8)
    t, out_ap = _declare_inputs(nc)
    with tile.TileContext(nc) as tc:
        _kernel(tc, t, out_ap, sim_compat)
    nc.compile()
    _CACHE[key] = nc
    return nc


def _kernel(tc, t, out_ap, sim_compat=False):
    nc = tc.nc
    import contextlib
    es = contextlib.ExitStack()

    eh_groups = [[0, 1], [2, 3], [4, 5], [6, 7]]       # d_inner-half pairs
    dir_groups = [[0, 2], [1, 3], [4, 6], [5, 7]]      # fwd/bwd pairs

    pers = es.enter_context(tc.tile_pool(name="pers", bufs=1))
    wp = es.enter_context(tc.tile_pool(name="wp", bufs=1))
    ck = es.enter_context(tc.tile_pool(name="ck", bufs=2))
    usb = es.enter_context(tc.tile_pool(name="usb", bufs=1))
    tp = es.enter_context(tc.tile_pool(name="tp", bufs=2))
    scn = es.enter_context(tc.tile_pool(name="scn", bufs=2))
    pm = es.enter_context(tc.tile_pool(name="pm", bufs=2, space="PSUM"))
    psY = es.enter_context(tc.tile_pool(name="psY", bufs=1, space="PSUM"))
    ps = es.enter_context(tc.tile_pool(name="ps", bufs=2, space="PSUM"))
    dram = es.enter_context(tc.tile_pool(name="dram", bufs=2, space="DRAM"))

    def apply_silu(out, psum_in, bias, uid):
        if not sim_compat:
            if bias is None:
                nc.scalar.activation(out, psum_in, AF.Silu)
            else:
                nc.scalar.activation(out, psum_in, AF.Silu, bias=bias)
            return
        tsg = tp.tile([P, Q], F32, tag="bcr", bufs=2, name=f"sg{uid}")
        tli = tp.tile([P, Q], F32, tag="bcr", bufs=2, name=f"sl{uid}")
        w = psum_in.shape[-1]
        if bias is None:
            nc.scalar.activation(tsg[:, 0:w], psum_in, AF.Sigmoid)
            nc.scalar.activation(tli[:, 0:w], psum_in, AF.Identity)
        else:
            nc.scalar.activation(tsg[:, 0:w], psum_in, AF.Sigmoid, bias=bias)
            nc.scalar.activation(tli[:, 0:w], psum_in, AF.Identity, bias=bias)
        nc.vector.tensor_mul(out, tli[:, 0:w], tsg[:, 0:w])

    # ---- persistent tiles
    x = [pers.tile([P, N], F32, tag=f"x{i}", name=f"x{i}") for i in range(KD)]
    ident_d = pers.tile([P, P], F32, tag="identd", name="ident_d")
    ident = pers.tile([P, P], F32, tag="ident", name="ident")
    ones_d = pers.tile([P, 1], F32, tag="onesd", name="ones_d")
    ones_col = pers.tile([P, 1], F32, tag="ones", name="ones_col")
    ones_bf = pers.tile([P, 1], BF16, tag="onesb", name="ones_bf")
    identb = pers.tile([P, P], BF16, tag="identb", name="identb")
    carry = pers.tile([P, NU * SG], BF16, tag="carry", name="carry")
    nc.sync.dma_start(ident_d[:], t["ident"])
    nc.vector.tensor_copy(ident[:], ident_d[:])
    nc.vector.tensor_copy(identb[:], ident_d[:])
    nc.sync.dma_start(ones_d[:], t["ones_col"])
    nc.vector.tensor_copy(ones_col[:], ones_d[:])
    nc.vector.tensor_copy(ones_bf[:], ones_d[:])

    # ---- load x directly in F-layout (host pre-transposed)
    for j in range(KD):
        nc.sync.dma_start(x[j][:], t["x_in"][j * P:(j + 1) * P, :])

    LS = {}   # per-layer state: weights, chunk tiles, dram tiles

    def load_weights(l):
        st = {}
        winw = [wp.tile([P, 2 * EL], BF16, tag=f"win{k}", name=f"win{l}_{k}")
                for k in range(KD)]
        for k in range(KD):
            nc.sync.dma_start(winw[k][:], t["win"][l, k * P:(k + 1) * P, :])
        convw = [[wp.tile([P, P], BF16, tag=f"cv{j}_{m}", name=f"cv{l}_{j}_{m}")
                  for m in range(ET)] for j in range(K)]
        for j in range(K):
            for m in range(ET):
                nc.sync.dma_start(convw[j][m][:], t["convd"][l, j, m])
        wxw = [wp.tile([P, RK + 2 * S], BF16, tag=f"wx{k}", name=f"wx{l}_{k}")
               for k in range(ET)]
        for k in range(ET):
            nc.sync.dma_start(wxw[k][:], t["wx"][l, k * P:(k + 1) * P, :])
        wdtw = wp.tile([RK, EL], BF16, tag="wdt", name=f"wdt{l}")
        nc.sync.dma_start(wdtw[:], t["wdt"][l])
        woutw = [wp.tile([P, DM], BF16, tag=f"wo{k}", name=f"wo{l}_{k}")
                 for k in range(ET)]
        for k in range(ET):
            nc.sync.dma_start(woutw[k][:], t["wout"][l, k * P:(k + 1) * P, :])
        amat = [wp.tile([P, S], F32, tag=f"am{m}", name=f"am{l}_{m}")
                for m in range(ET)]
        for m in range(ET):
            nc.sync.dma_start(amat[m][:], t["amat"][l, m * P:(m + 1) * P, :])
        pcol = [[wp.tile([P, 1], F32, tag=f"pc{w}_{m}", name=f"pc{w}{l}_{m}")
                 for m in range(ET)] for w in range(3)]
        for m in range(ET):
            sl = slice(m * P, (m + 1) * P)
            nc.sync.dma_start(pcol[0][m][:], t["bdt"][l, sl].unsqueeze(-1))
            nc.sync.dma_start(pcol[1][m][:], t["conv_b"][l, sl].unsqueeze(-1))
            nc.sync.dma_start(pcol[2][m][:], t["dvec"][l, sl].unsqueeze(-1))
        gcol = [wp.tile([P, 1], F32, tag=f"gc{i}", name=f"gc{l}_{i}")
                for i in range(KD)]
        bcol = [wp.tile([P, 1], F32, tag=f"bc{i}", name=f"bc{l}_{i}")
                for i in range(KD)]
        for i in range(KD):
            sl = slice(i * P, (i + 1) * P)
            nc.sync.dma_start(gcol[i][:], t["ln_g"][l, sl].unsqueeze(-1))
            nc.sync.dma_start(bcol[i][:], t["ln_b"][l, sl].unsqueeze(-1))
        st.update(winw=winw, convw=convw, wxw=wxw, wdtw=wdtw, woutw=woutw,
                  amat=amat, bdtc=pcol[0], cbc=pcol[1], dvc=pcol[2],
                  gcol=gcol, bcol=bcol, usb=[None] * ET, ch={})
        LS[l] = st

    def emit_A(l, c):
        """LN -> Win-u -> conv -> silu -> Wx -> AllReduce -> z -> dt -> vb
        for chunk c of layer l."""
        st = LS[l]
        cs = slice(c * Q, (c + 1) * Q)
        uid = f"{l}_{c}"
        # -- LN stats over the feature (partition) axis
        sxq = ps.tile([33, Q], F32, tag="st", name=f"sxq{uid}")
        sx, sq = sxq[0:1], sxq[32:33]
        for i in range(KD):
            xsq = tp.tile([P, Q], BF16, tag="xsqb", bufs=2, name=f"xsq{uid}_{i}")
            nc.scalar.square(xsq[:], x[i][:, cs])
            nc.tensor.matmul(sx[:], ones_col[:], x[i][:, cs],
                             start=(i == 0), stop=(i == KD - 1))
            nc.tensor.matmul(sq[:], ones_bf[:], xsq[:],
                             start=(i == 0), stop=(i == KD - 1))
        nm = ck.tile([1, Q], BF16, tag="nm", name=f"nm{uid}")
        rstd = ck.tile([1, Q], BF16, tag="rstd", name=f"rstd{uid}")
        nc.vector.tensor_scalar_mul(nm[0:1], sx[:], -1.0 / DM)
        nc.vector.tensor_scalar_mul(rstd[0:1], sq[:], 1.0 / DM)
        nc.vector.tensor_mul(sq[:], nm[0:1], nm[0:1])
        nc.vector.tensor_sub(rstd[0:1], rstd[0:1], sq[:])
        nc.vector.tensor_scalar_add(rstd[0:1], rstd[0:1], 1e-5)
        nc.scalar.activation(rstd[0:1], rstd[0:1], AF.Ln)
        nc.scalar.activation(rstd[0:1], rstd[0:1], AF.Exp, scale=-0.5)
        nmb = ck.tile([P, Q], BF16, tag="nmb", name=f"nmb{uid}")
        rsb = ck.tile([P, Q], BF16, tag="rsb", name=f"rsb{uid}")
        nc.gpsimd.partition_broadcast(nmb[:], nm[0:1])
        nc.gpsimd.partition_broadcast(rsb[:], rstd[0:1])
        hln = [ck.tile([P, Q], BF16, tag=f"hln{i}", name=f"hln{uid}_{i}")
               for i in range(KD)]
        for i in range(KD):
            t1 = tp.tile([P, Q], F32, tag="t4", bufs=2, name=f"lnt1_{uid}_{i}")
            nc.vector.tensor_add(t1[:], x[i][:, cs], nmb[:])
            nc.vector.tensor_mul(t1[:], t1[:], rsb[:])
            nc.vector.tensor_scalar(hln[i][:], t1[:], st["gcol"][i][:],
                                    st["bcol"][i][:], op0=OP.mult, op1=OP.add)
        # -- Win u-wave + conv + silu
        usi = [ck.tile([P, Q], BF16, tag=f"usi{m}", name=f"usi{uid}_{m}")
               for m in range(ET)]
        for m in range(ET):
            pu = pm.tile([P, Q], F32, tag="gemm", name=f"pu{uid}_{m}")
            for k in range(KD):
                nc.tensor.matmul(pu[:], st["winw"][k][:, m * P:(m + 1) * P],
                                 hln[k][:], start=(k == 0), stop=(k == KD - 1))
            if c == 0:
                st["usb"][m] = usb.tile([P, K - 1 + N], BF16, tag=f"usb{m}",
                                        name=f"usb{l}_{m}")
                nc.vector.memset(st["usb"][m][:, 0:K - 1], 0.0)
            u_sb = st["usb"][m]
            nc.scalar.copy(u_sb[:, K - 1 + c * Q:K - 1 + (c + 1) * Q], pu[:])
            pc = pm.tile([P, Q], F32, tag="gemm", name=f"pcv{uid}_{m}")
            for j in range(K):
                nc.tensor.matmul(pc[:], st["convw"][j][m][:],
                                 u_sb[:, c * Q + j:c * Q + j + Q],
                                 start=(j == 0), stop=(j == K - 1))
            apply_silu(usi[m][:], pc[:], st["cbc"][m][:], f"u{uid}_{m}")
        # -- Wx partial GEMM + pair AllReduce (bf16)
        px = pm.tile([P, Q], F32, tag="gemm", name=f"px{uid}")
        for k in range(ET):
            nc.tensor.matmul(px[0:RK + 2 * S, :], st["wxw"][k][:], usi[k][:],
                             start=(k == 0), stop=(k == ET - 1))
        xdp = dram.tile([RK + 2 * S, Q], BF16, tag="xdp", name=f"xdp{uid}")
        xds = dram.tile([RK + 2 * S, Q], BF16, tag="xds", name=f"xds{uid}")
        pxs = tp.tile([RK + 2 * S, Q], BF16, tag="pxsb", bufs=2,
                      name=f"pxs{uid}")
        nc.scalar.copy(pxs[:], px[0:RK + 2 * S, :])
        nc.sync.dma_start(xdp[:], pxs[:])
        nc.gpsimd.collective_compute(
            "AllReduce", OP.add, replica_groups=eh_groups,
            ins=[xdp[:]], outs=[xds[:]])
        # -- Win z-wave + silu (overlaps the collective)
        zsi = [ck.tile([P, Q], BF16, tag=f"zsi{m}", name=f"zsi{uid}_{m}")
               for m in range(ET)]
        for m in range(ET):
            pz = pm.tile([P, Q], F32, tag="gemm", name=f"pz{uid}_{m}")
            for k in range(KD):
                nc.tensor.matmul(pz[:],
                                 st["winw"][k][:, (ET + m) * P:(ET + m + 1) * P],
                                 hln[k][:], start=(k == 0), stop=(k == KD - 1))
            apply_silu(zsi[m][:], pz[:], None, f"z{uid}_{m}")
        st["ch"][c] = dict(usi=usi, zsi=zsi, xds=xds)

    def emit_A_dt(l, c):
        """dt = softplus(xdbl[:,:RK] @ Wdt + bdt).  Emitted well after the
        Wx AllReduce was issued, so the scalar queue is not convoyed behind
        the collective round trip."""
        st = LS[l]
        chs = st["ch"][c]
        uid = f"{l}_{c}"
        xdbl_bf = ck.tile([RK, Q], BF16, tag="xdblb", name=f"xdblb{uid}")
        nc.sync.dma_start(xdbl_bf[:], chs["xds"][0:RK, :])
        dtb = [ck.tile([P, Q], BF16, tag=f"dtb{m}", name=f"dtb{uid}_{m}")
               for m in range(ET)]
        spxs = []
        for m in range(ET):
            pd = pm.tile([P, Q], F32, tag="gemm", name=f"pd{uid}_{m}")
            nc.tensor.matmul(pd[:], st["wdtw"][:, m * P:(m + 1) * P],
                             xdbl_bf[:], start=True, stop=True)
            spx = tp.tile([P, Q], BF16, tag="spx", bufs=4, name=f"spx{uid}_{m}")
            nc.scalar.activation(spx[:], pd[:], AF.Exp, bias=st["bdtc"][m][:])
            spxs.append(spx)
        for m in range(ET):
            nc.scalar.activation(dtb[m][:], spxs[m][:], AF.Ln,
                                 bias=ones_col[:])
        chs["dtb"] = dtb

    def emit_B(l, c, embeds):
        """Scan units for chunk c of layer l; embeds[u] callables are
        emitted after unit u (to thread other chunks' work through the
        DVE-paced stream)."""
        st = LS[l]
        chs = st["ch"][c]
        vb = [ck.tile([P, Q], BF16, tag=f"vb{m}", name=f"vb{l}_{c}_{m}")
              for m in range(ET)]
        for m in range(ET):
            nc.vector.tensor_mul(vb[m][:], chs["dtb"][m][:],
                                 chs["usi"][m][:])
        chs["vb"] = vb
        yacc = [psY.tile([P, Q], F32, tag=f"yac{m}", name=f"yac{l}_{c}_{m}")
                for m in range(ET)]
        chs["yacc"] = yacc
        for g in range(NG):
            bb = scn.tile([P, GF], BF16, tag="bb", name=f"bb{l}_{c}_{g}")
            cc = scn.tile([P, GF], BF16, tag="cc", name=f"cc{l}_{c}_{g}")
            nc.sync.dma_start(
                bb[:],
                chs["xds"][RK + g * SG:RK + (g + 1) * SG,
                           :].partition_broadcast(P))
            nc.sync.dma_start(
                cc[:],
                chs["xds"][RK + S + g * SG:RK + S + (g + 1) * SG,
                           :].partition_broadcast(P))
            bb3 = bb.rearrange("p (s n) -> p s n", s=SG)
            for m in range(ET):
                u = g * ET + m
                uid = f"{l}_{c}_{g}_{m}"
                da = scn.tile([P, GF], BF16, tag="da", name=f"da{uid}")
                for si in range(SG):
                    s = g * SG + si
                    nc.scalar.activation(da[:, si * Q:(si + 1) * Q],
                                         chs["dtb"][m][:], AF.Exp,
                                         scale=st["amat"][m][:, s:s + 1])
                da3 = da.rearrange("p (s n) -> p s n", s=SG)
                dbu = scn.tile([P, GF], BF16, tag="dbu", bufs=2,
                               name=f"dbu{uid}")
                vv = chs["vb"][m].unsqueeze(1).broadcast_to((P, SG, Q))
                db3 = dbu.rearrange("p (s n) -> p s n", s=SG)
                nc.vector.tensor_mul(db3[:], vv, bb3[:])
                if c > 0:
                    # fold the cross-chunk carry into dbu[., si, 0]
                    for si in range(SG):
                        col = u * SG + si
                        nc.vector.scalar_tensor_tensor(
                            db3[:, si, 0:1], da3[:, si, 0:1],
                            carry[:, col:col + 1], db3[:, si, 0:1],
                            op0=OP.mult, op1=OP.add)
                nc.vector.memset(da3[:, :, 0:1], 0.0)
                hh = scn.tile([P, GF], BF16, tag="dbu", bufs=2,
                              name=f"hh{uid}")
                nc.vector.tensor_tensor_scan(hh[:], da[:], dbu[:], 0.0,
                                             op0=OP.mult, op1=OP.add)
                if c < CH - 1:
                    hh3 = hh.rearrange("p (s n) -> p s n", s=SG)
                    nc.vector.tensor_copy(
                        carry[:, u * SG:(u + 1) * SG], hh3[:, :, Q - 1])
                ym = scn.tile([P, GF], BF16, tag="ymt", bufs=2,
                              name=f"ym{uid}")
                nc.vector.tensor_mul(ym[:], hh[:], cc[:])
                for si in range(SG):
                    nc.tensor.matmul(yacc[m][:], identb[:],
                                     ym[:, si * Q:(si + 1) * Q],
                                     start=(g == 0 and si == 0),
                                     stop=(g == NG - 1 and si == SG - 1))
                if g == NG - 1:
                    # gate m as soon as its yacc closes; Wout k-major so
                    # its first matmuls overlap the remaining scan units
                    gated = chs.setdefault("gated", [None] * ET)
                    gated[m] = ck.tile([P, Q], BF16, tag=f"gt{m}",
                                       name=f"gt{l}_{c}_{m}")
                    nc.vector.scalar_tensor_tensor(
                        gated[m][:], chs["usi"][m][:], st["dvc"][m][:],
                        yacc[m][:], op0=OP.mult, op1=OP.add)
                    nc.vector.tensor_mul(gated[m][:], gated[m][:],
                                         chs["zsi"][m][:])
                for fn in embeds.get(u, ()):
                    fn()

    def emit_C(l, c):
        """Wout partial + pair AllReduce for chunk c (residual deferred)."""
        st = LS[l]
        chs = st["ch"][c]
        uid = f"{l}_{c}"
        dxp = dram.tile([DM, Q], BF16, tag="dxp", name=f"dxp{uid}")
        dxs = dram.tile([DM, Q], BF16, tag="dxs", name=f"dxs{uid}")
        chs["dxs"] = dxs
        po = [psY.tile([P, Q], F32, tag=f"yac{mo}", name=f"po{uid}_{mo}")
              for mo in range(KD)]
        for k in range(ET):
            for mo in range(KD):
                nc.tensor.matmul(po[mo][:],
                                 st["woutw"][k][:, mo * P:(mo + 1) * P],
                                 chs["gated"][k][:],
                                 start=(k == 0), stop=(k == ET - 1))
        for mo in range(KD):
            pos = tp.tile([P, Q], BF16, tag="bpd", bufs=2,
                          name=f"pos{uid}_{mo}")
            nc.scalar.copy(pos[:], po[mo][:])
            nc.sync.dma_start(dxp[mo * P:(mo + 1) * P, :], pos[:])
            if mo % 2 == 1:
                nc.gpsimd.collective_compute(
                    "AllReduce", OP.add, replica_groups=eh_groups,
                    ins=[dxp[(mo - 1) * P:(mo + 1) * P, :]],
                    outs=[dxs[(mo - 1) * P:(mo + 1) * P, :]])

    def emit_resid(l, c, mos):
        st = LS[l]
        chs = st["ch"][c]
        cs = slice(c * Q, (c + 1) * Q)
        for mo in mos:
            dxt = tp.tile([P, Q], BF16, tag="bpd", bufs=2,
                          name=f"dxt{l}_{c}_{mo}")
            nc.gpsimd.dma_start(dxt[:], chs["dxs"][mo * P:(mo + 1) * P, :])
            nc.vector.tensor_add(x[mo][:, cs], x[mo][:, cs], dxt[:])

    # ================= combine helpers =================
    DM2 = DM + 2
    cat_part = [dram.tile([DM2, Q], BF16, tag=f"catp{c}", bufs=1,
                          name=f"cat_part{c}") for c in range(CH)]
    cat_sum = [dram.tile([2 * DM2, Q], BF16, tag=f"cats{c}", bufs=1,
                         name=f"cat_sum{c}") for c in range(CH)]

    def emit_cat(c):
        """Ship our direction's chunk-c output (straight, no flip) plus its
        LN partial stats, then AllGather the dir pair."""
        cs = slice(c * Q, (c + 1) * Q)
        sxq = ps.tile([33, Q], F32, tag="st", name=f"csxq{c}")
        for i in range(KD):
            sf = tp.tile([P, Q], BF16, tag="bpd", bufs=2, name=f"sf{c}_{i}")
            nc.vector.tensor_copy(sf[:], x[i][:, cs])
            nc.sync.dma_start(cat_part[c][i * P:(i + 1) * P, :], sf[:])
            xsq = tp.tile([P, Q], BF16, tag="xsqb", bufs=2, name=f"pxq{c}_{i}")
            nc.scalar.square(xsq[:], sf[:])
            nc.tensor.matmul(sxq[0:1, :], ones_bf[:], sf[:],
                             start=(i == 0), stop=(i == KD - 1))
            nc.tensor.matmul(sxq[32:33, :], ones_bf[:], xsq[:],
                             start=(i == 0), stop=(i == KD - 1))
        stats_sb = tp.tile([33, Q], BF16, tag="cstat", bufs=2,
                           name=f"stats_sb{c}")
        nc.scalar.copy(stats_sb[0:1, :], sxq[0:1, :])
        nc.scalar.copy(stats_sb[32:33, :], sxq[32:33, :])
        nc.sync.dma_start(cat_part[c][DM:DM + 1, :], stats_sb[0:1, :])
        nc.sync.dma_start(cat_part[c][DM + 1:DM + 2, :], stats_sb[32:33, :])
        nc.gpsimd.collective_compute(
            "AllGather", OP.bypass, replica_groups=dir_groups,
            ins=[cat_part[c][:]], outs=[cat_sum[c][:]])

    # ================= emission schedule =================
    load_weights(0)
    # combine weights prefetch (idle DMA time at the start)
    cww = [wp.tile([P, DM], BF16, tag=f"cwt{k}", name=f"cw{k}")
           for k in range(2 * KD)]
    for k in range(2 * KD):
        nc.sync.dma_start(cww[k][:], t["cw"][k * P:(k + 1) * P, :])
    cbias_c = [wp.tile([P, 1], F32, tag=f"cbs{m}", name=f"cbs{m}")
               for m in range(KD)]
    for m in range(KD):
        nc.sync.dma_start(cbias_c[m][:],
                          t["cbias"][m * P:(m + 1) * P].unsqueeze(-1))
    cgcol = [wp.tile([P, 1], F32, tag="cgcol", bufs=8, name=f"cgc{i}")
             for i in range(2 * KD)]
    cbcol = [wp.tile([P, 1], F32, tag="cbcol", bufs=8, name=f"cbc{i}")
             for i in range(2 * KD)]
    for i in range(2 * KD):
        nc.sync.dma_start(cgcol[i][:], t["cg"][i * P:(i + 1) * P].unsqueeze(-1))
        nc.sync.dma_start(cbcol[i][:], t["cb"][i * P:(i + 1) * P].unsqueeze(-1))
    emit_A(0, 0)
    emit_A_dt(0, 0)
    for l in range(L):
        embeds0 = {}
        if l > 0:
            embeds0[2] = [lambda l=l: emit_resid(l - 1, 1, (0, 1))]
            embeds0[3] = [lambda l=l: emit_resid(l - 1, 1, (2, 3))]
        embeds0[4] = [lambda l=l: emit_A(l, 1)]
        embeds0[10] = [lambda l=l: emit_A_dt(l, 1)]
        emit_B(l, 0, embeds0)
        emit_C(l, 0)
        embeds1 = {
            2: [lambda l=l: emit_resid(l, 0, (0, 1))],
            3: [lambda l=l: emit_resid(l, 0, (2, 3))],
        }
        if l < L - 1:
            embeds1[4] = [lambda l=l: (load_weights(l + 1),
                                       emit_A(l + 1, 0))]
            embeds1[10] = [lambda l=l: emit_A_dt(l + 1, 0)]
        else:
            embeds1[5] = [lambda: emit_cat(0)]
        emit_B(l, 1, embeds1)
        emit_C(l, 1)
    emit_resid(L - 1, 1, (0, 1, 2, 3))
    emit_cat(1)

    # ================= combine =================
    def cat_row(i, c):
        """Feature-tile i of chunk c of the concat layout."""
        if i < KD:
            return cat_sum[c][i * P:(i + 1) * P, :]
        return cat_sum[c][DM2 + (i - KD) * P:DM2 + (i - KD + 1) * P, :]

    # global LN stats in OUTPUT token order: fwd stats straight + bwd
    # stats column-reversed (bwd stream position p holds token N-1-p).
    cnm = ck.tile([1, N], BF16, tag="cnm", name="cnm")
    crstd = ck.tile([1, N], BF16, tag="crstd", name="crstd")
    sxb = ck.tile([1, N], BF16, tag="sxb", name="sxb")
    sqb = ck.tile([1, N], BF16, tag="sqb", name="sqb")
    for c in range(CH):
        cs = slice(c * Q, (c + 1) * Q)
        nc.sync.dma_start(cnm[0:1, cs], cat_sum[c][DM:DM + 1, :])
        nc.sync.dma_start(crstd[0:1, cs], cat_sum[c][DM + 1:DM + 2, :])
        nc.sync.dma_start(sxb[0:1, cs], cat_sum[c][DM2 + DM:DM2 + DM + 1, :])
        nc.sync.dma_start(sqb[0:1, cs],
                          cat_sum[c][DM2 + DM + 1:DM2 + DM + 2, :])
    nc.vector.tensor_add(cnm[0:1, :], cnm[0:1, :], sxb[0:1, ::-1])
    nc.vector.tensor_add(crstd[0:1, :], crstd[0:1, :], sqb[0:1, ::-1])
    nc.vector.tensor_scalar_mul(cnm[0:1, :], cnm[0:1, :], -1.0 / (2 * DM))
    nc.vector.tensor_scalar_mul(crstd[0:1, :], crstd[0:1, :], 1.0 / (2 * DM))
    nc.vector.tensor_mul(sxb[0:1, :], cnm[0:1, :], cnm[0:1, :])
    nc.vector.tensor_sub(crstd[0:1, :], crstd[0:1, :], sxb[0:1, :])
    nc.vector.tensor_scalar_add(crstd[0:1, :], crstd[0:1, :], 1e-5)
    nc.scalar.activation(crstd[0:1, :], crstd[0:1, :], AF.Ln)
    nc.scalar.activation(crstd[0:1, :], crstd[0:1, :], AF.Exp, scale=-0.5)
    cnmb = ck.tile([P, N], BF16, tag="cnmb", bufs=1, name="cnmb")
    crsb = ck.tile([P, N], BF16, tag="crsb", bufs=1, name="crsb")
    nc.gpsimd.partition_broadcast(cnmb[:], cnm[0:1, :])
    nc.gpsimd.partition_broadcast(crsb[:], crstd[0:1, :])

    for q in range(N // CQ):
        qs = slice(q * CQ, (q + 1) * CQ)
        rq = N - (q + 1) * CQ      # bwd source cols (to be read reversed)
        xc = [ck.tile([P, CQ], BF16, tag="xc", bufs=9, name=f"xc{q}_{i}")
              for i in range(2 * KD)]
        for i in range(2 * KD):
            if i < KD:
                c0, o0 = divmod(q * CQ, Q)
                src = cat_row(i, c0)[:, o0:o0 + CQ]
            else:
                c0, o0 = divmod(rq, Q)
                src = cat_row(i, c0)[:, o0:o0 + CQ]
            nc.sync.dma_start(xc[i][:], src)
        hcq = [ck.tile([P, CQ], BF16, tag="hc", bufs=9, name=f"hc{q}_{i}")
               for i in range(2 * KD)]
        for i in range(2 * KD):
            xin = xc[i][:, :] if i < KD else xc[i][:, ::-1]
            t1c = tp.tile([P, CQ], F32, tag="lnt1c", bufs=2, name=f"t1c{q}_{i}")
            nc.vector.tensor_add(t1c[:], xin, cnmb[:, qs])
            nc.vector.tensor_mul(t1c[:], t1c[:], crsb[:, qs])
            nc.vector.tensor_scalar(hcq[i][:], t1c[:], cgcol[i][:],
                                    cbcol[i][:], op0=OP.mult, op1=OP.add)
        ot = tp.tile([P, DM], F32, tag="tio", bufs=2, name=f"ot{q}_a")
        ot2 = tp.tile([P, DM], F32, tag="tio", bufs=2, name=f"ot{q}_b")
        for m in range(KD):
            pg = pm.tile([P, Q], F32, tag="gemm", name=f"pg{q}_{m}")
            for k in range(2 * KD):
                nc.tensor.matmul(pg[:, 0:CQ], cww[k][:, m * P:(m + 1) * P],
                                 hcq[k][:], start=(k == 0),
                                 stop=(k == 2 * KD - 1))
            ogm = tp.tile([P, CQ], F32, tag="og", bufs=2, name=f"og{q}_{m}")
            gfn = AF.Identity if sim_compat else AF.Gelu
            nc.scalar.activation(ogm[:, 0:CQ], pg[:, 0:CQ], gfn,
                                 bias=cbias_c[m][:])
            for hh2 in range(CQ // P):
                pts = pm.tile([P, Q], F32, tag="gemm", name=f"otp{q}_{m}_{hh2}")
                nc.tensor.transpose(
                    pts[:, 0:P], ogm[:, hh2 * P:(hh2 + 1) * P], ident[:])
                dst = ot if hh2 == 0 else ot2
                nc.scalar.copy(dst[:, m * P:(m + 1) * P], pts[:, 0:P])
        nc.sync.dma_start(out_ap[q * CQ:q * CQ + P, :], ot[:])
        nc.sync.dma_start(out_ap[q * CQ + P:(q + 1) * CQ, :], ot2[:])

    es.close()


# ----------------------------------------------------------------- host side
def _bf(a):
    return np.asarray(a, dtype=np.float32).astype(ml_dtypes.bfloat16)


def _core_inputs(inputs, b, dirn, e):
    pre = "fwd" if dirn == 0 else "bwd"
    g = lambda n: np.asarray(inputs[pre + "_" + n], dtype=np.float32)
    x = np.asarray(inputs["x"], dtype=np.float32)[b]          # (N, DM)
    if dirn == 1:
        x = x[::-1]
    es = slice(e * EL, (e + 1) * EL)

    win_full = g("Win")                                        # (L, DM, 2*EFULL)
    win = np.concatenate(
        [win_full[:, :, e * EL:(e + 1) * EL],
         win_full[:, :, EFULL + e * EL:EFULL + (e + 1) * EL]], axis=2)

    cw4 = g("conv_w")[:, es, 0, :]                             # (L, EL, K)
    convd = np.zeros((L, K, ET, P, P), np.float32)
    for j in range(K):
        for m in range(ET):
            for l in range(L):
                np.fill_diagonal(convd[l, j, m], cw4[l, m * P:(m + 1) * P, j])

    return {
        "x_in": np.ascontiguousarray(x.T),
        "ident": np.eye(P, dtype=np.float32),
        "ones_col": np.ones((P, 1), np.float32),
        "ln_g": g("ln_g"), "ln_b": g("ln_b"),
        "win": _bf(win),
        "convd": _bf(convd),
        "conv_b": g("conv_b")[:, es],
        "wx": _bf(g("Wx")[:, es, :]),
        "wdt": _bf(g("Wdt")[:, :, es]),
        "bdt": g("bdt")[:, es],
        "amat": -np.exp(g("Alog")[:, es, :]),
        "dvec": g("D")[:, es],
        "wout": _bf(g("Wout")[:, es, :]),
        "cg": np.asarray(inputs["cmb_ln_g"], np.float32),
        "cb": np.asarray(inputs["cmb_ln_b"], np.float32),
        "cw": _bf(np.asarray(inputs["cmb_W"], np.float32)),
        "cbias": np.asarray(inputs["cmb_b"], np.float32),
    }


def make_in_maps(inputs):
    in_maps = []
    for b in range(B):
        for dirn in range(2):
            for e in range(2):
                in_maps.append(_core_inputs(inputs, b, dirn, e))
    return in_maps


def kernel(**inputs):
    nc = _build_program()
    res = bass_utils.run_bass_kernel_spmd(nc, make_in_maps(inputs),
                                          list(range(8)))
    out = np.empty((B, N, DM), np.float32)
    for b in range(B):
        out[b] = res.results[b * 4]["out"]
    return out


if __name__ == "__main__":
    nc = _build_program()
    n_inst = sum(len(bb.instructions) for f in nc.m.functions for bb in f.blocks)
    print("program built ok:", n_inst, "instructions")


# revision 23
# speedup vs baseline: 1.0189x; 1.0012x over previous
"""Bidirectional MAMBA Trainium2 kernel, token-chunk software-pipelined.

Sharding (8 cores): (batch 2) x (direction 2) x (d_inner half 2).
Each core runs the full 4-layer chain of its direction on its batch with
E_loc=512 of the 1024 d_inner channels over the full N=1024 sequence
(backward stream is pre-flipped on the host).  Two pair-AllReduces per
layer chunk combine the u@Wx partials and the y@Wout partials.  The
selective scan runs exactly on the DVE via tensor_tensor_scan with S
state channels glued along the free dimension (dA zeroed at segment
starts resets the state), channels on partitions.

Pipelining: each layer is processed in CH=2 token chunks of Q=512.  The
DVE is the bottleneck engine (scan + elementwise ~230us/layer), so the
emission order threads everything else through the scan stream: while
the DVE scans chunk c, the tensor/scalar/CC engines run the Wout+AllReduce
+residual of chunk c-1 and the LN/Win/conv/Wx/AllReduce/dt prep of chunk
c+1, embedded at unit boundaries of the scan loop.  Scan state crosses
the chunk boundary via a per-unit carry column folded into dbu[.,si,0]
(h[0] = da[0]*h_in + dbu[0]) with da[.,si,0] zeroed afterwards.

Perf notes inherited from the unpipelined version: collectives in bf16;
LN stats broadcast via gpsimd partition_broadcast; activation-table
loads batched; scan-phase elementwise stays on the DVE (gpsimd Pool
measured ~3.1ns/elem and its SBUF traffic slows concurrent DVE scans
~2x).  The final direction-concat AllGather ships per chunk without the
flip (the combine reads backward-direction rows through reversed APs).

Self-contained: hardcodes all shapes; only needs trn_rl_repo on sys.path.
"""

import os
import sys

import numpy as np

for _p in ("/opt/trn_rl_repo", "/root/.axon_site/_ro/trn_rl_repo"):
    if os.path.isdir(_p) and _p not in sys.path:
        sys.path.insert(0, _p)

import ml_dtypes  # noqa: E402

import concourse.bacc as bacc  # noqa: E402
import concourse.bass as bass  # noqa: E402
import concourse.mybir as mybir  # noqa: E402
import concourse.tile as tile  # noqa: E402
from concourse import bass_utils, library_config  # noqa: E402

F32 = mybir.dt.float32
BF16 = mybir.dt.bfloat16
AF = mybir.ActivationFunctionType
OP = mybir.AluOpType

B, N, DM = 2, 1024, 512          # batch, seq, d_model
EFULL, S, RK, L, K = 1024, 16, 32, 4, 4  # d_inner, d_state, dt_rank, layers, conv
EL = EFULL // 2                  # d_inner per core (tensor-parallel half)
P = 128
KD = DM // P                     # 4 k-tiles over d_model
ET = EL // P                     # 4 tiles over local d_inner
CH = 2                           # token chunks per layer
Q = N // CH                      # tokens per chunk (512)
SG = 4                           # s-values glued per scan unit
NG = S // SG                     # 4 scan groups per chunk
GF = SG * Q                      # glued free size per scan unit (2048)
NU = NG * ET                     # 16 scan units per chunk
CQ = 256                         # combine token-chunk

_CACHE = {}


def _patch_act_tables():
    """Leave `natural_log_exp_and_others` as the only table providing Exp
    and Ln, so the act-table placement pass stops bouncing between the
    exp-only and ln-only sets (each reload costs ~1.3us of scalar time).
    Set ids are positional, so entries are edited in place, never
    reordered."""
    if _CACHE.get("actpatch"):
        return
    import functools
    import concourse.hw_specs as hw_specs
    import concourse.bacc as bacc_mod
    orig = hw_specs.get_activation_tables

    @functools.cache
    def patched(arch):
        tabs = {k: set(v) for k, v in orig(arch).items()}
        for name, fns in tabs.items():
            if name != "natural_log_exp_and_others":
                fns.discard(AF.Exp)
                fns.discard(AF.Ln)
        return tabs

    hw_specs.get_activation_tables = patched
    bacc_mod.get_activation_tables = patched
    _CACHE["actpatch"] = True


def _declare_inputs(nc):
    t = {}

    def d(name, shape, dt=F32):
        t[name] = nc.dram_tensor(name, list(shape), dt, kind="ExternalInput").ap()

    d("x_in", (DM, N))      # host passes x pre-transposed (F-layout)
    d("ident", (P, P))
    d("ones_col", (P, 1))
    d("ln_g", (L, DM)); d("ln_b", (L, DM))
    d("win", (L, DM, 2 * EL), BF16)      # cols = [u-half | z-half]
    d("convd", (L, K, ET, P, P), BF16)   # diagonalized depthwise conv weights
    d("conv_b", (L, EL))
    d("wx", (L, EL, RK + 2 * S), BF16)
    d("wdt", (L, RK, EL), BF16)
    d("bdt", (L, EL))
    d("amat", (L, EL, S))                # A = -exp(Alog) slice
    d("dvec", (L, EL))
    d("wout", (L, EL, DM), BF16)
    d("cg", (2 * DM,)); d("cb", (2 * DM,))
    d("cw", (2 * DM, DM), BF16)
    d("cbias", (DM,))
    out = nc.dram_tensor("out", [N, DM], F32, kind="ExternalOutput").ap()
    return t, out


def _build_program(sim_compat=False):
    key = ("prog", sim_compat)
    if key in _CACHE:
        return _CACHE[key]
    _patch_act_tables()
    nc = bacc.Bacc("TRN2", target_bir_lowering=False, debug=False, num_devices=8)
    t, out_ap = _declare_inputs(nc)
    with tile.TileContext(nc) as tc:
        _kernel(tc, t, out_ap, sim_compat)
    nc.compile()
    _CACHE[key] = nc
    return nc


def _kernel(tc, t, out_ap, sim_compat=False):
    nc = tc.nc
    import contextlib
    es = contextlib.ExitStack()

    eh_groups = [[0, 1], [2, 3], [4, 5], [6, 7]]       # d_inner-half pairs
    dir_groups = [[0, 2], [1, 3], [4, 6], [5, 7]]      # fwd/bwd pairs

    pers = es.enter_context(tc.tile_pool(name="pers", bufs=1))
    wp = es.enter_context(tc.tile_pool(name="wp", bufs=1))
    ck = es.enter_context(tc.tile_pool(name="ck", bufs=2))
    usb = es.enter_context(tc.tile_pool(name="usb", bufs=1))
    tp = es.enter_context(tc.tile_pool(name="tp", bufs=2))
    scn = es.enter_context(tc.tile_pool(name="scn", bufs=2))
    pm = es.enter_context(tc.tile_pool(name="pm", bufs=2, space="PSUM"))
    psY = es.enter_context(tc.tile_pool(name="psY", bufs=1, space="PSUM"))
    ps = es.enter_context(tc.tile_pool(name="ps", bufs=2, space="PSUM"))
    dram = es.enter_context(tc.tile_pool(name="dram", bufs=2, space="DRAM"))

    def apply_silu(out, psum_in, bias, uid):
        if not sim_compat:
            if bias is None:
                nc.scalar.activation(out, psum_in, AF.Silu)
            else:
                nc.scalar.activation(out, psum_in, AF.Silu, bias=bias)
            return
        tsg = tp.tile([P, Q], F32, tag="bcr", bufs=2, name=f"sg{uid}")
        tli = tp.tile([P, Q], F32, tag="bcr", bufs=2, name=f"sl{uid}")
        w = psum_in.shape[-1]
        if bias is None:
            nc.scalar.activation(tsg[:, 0:w], psum_in, AF.Sigmoid)
            nc.scalar.activation(tli[:, 0:w], psum_in, AF.Identity)
        else:
            nc.scalar.activation(tsg[:, 0:w], psum_in, AF.Sigmoid, bias=bias)
            nc.scalar.activation(tli[:, 0:w], psum_in, AF.Identity, bias=bias)
        nc.vector.tensor_mul(out, tli[:, 0:w], tsg[:, 0:w])

    # ---- persistent tiles
    x = [pers.tile([P, N], F32, tag=f"x{i}", name=f"x{i}") for i in range(KD)]
    ident_d = pers.tile([P, P], F32, tag="identd", name="ident_d")
    ident = pers.tile([P, P], F32, tag="ident", name="ident")
    ones_d = pers.tile([P, 1], F32, tag="onesd", name="ones_d")
    ones_col = pers.tile([P, 1], F32, tag="ones", name="ones_col")
    ones_bf = pers.tile([P, 1], BF16, tag="onesb", name="ones_bf")
    identb = pers.tile([P, P], BF16, tag="identb", name="identb")
    carry = pers.tile([P, NU * SG], BF16, tag="carry", name="carry")
    nc.sync.dma_start(ident_d[:], t["ident"])
    nc.vector.tensor_copy(ident[:], ident_d[:])
    nc.vector.tensor_copy(identb[:], ident_d[:])
    nc.sync.dma_start(ones_d[:], t["ones_col"])
    nc.vector.tensor_copy(ones_col[:], ones_d[:])
    nc.vector.tensor_copy(ones_bf[:], ones_d[:])

    # ---- load x directly in F-layout (host pre-transposed)
    for j in range(KD):
        nc.sync.dma_start(x[j][:], t["x_in"][j * P:(j + 1) * P, :])

    LS = {}   # per-layer state: weights, chunk tiles, dram tiles

    def load_weights(l):
        st = {}
        winw = [wp.tile([P, 2 * EL], BF16, tag=f"win{k}", name=f"win{l}_{k}")
                for k in range(KD)]
        for k in range(KD):
            nc.sync.dma_start(winw[k][:], t["win"][l, k * P:(k + 1) * P, :])
        convw = [[wp.tile([P, P], BF16, tag=f"cv{j}_{m}", name=f"cv{l}_{j}_{m}")
                  for m in range(ET)] for j in range(K)]
        for j in range(K):
            for m in range(ET):
                nc.sync.dma_start(convw[j][m][:], t["convd"][l, j, m])
        wxw = [wp.tile([P, RK + 2 * S], BF16, tag=f"wx{k}", name=f"wx{l}_{k}")
               for k in range(ET)]
        for k in range(ET):
            nc.sync.dma_start(wxw[k][:], t["wx"][l, k * P:(k + 1) * P, :])
        wdtw = wp.tile([RK, EL], BF16, tag="wdt", name=f"wdt{l}")
        nc.sync.dma_start(wdtw[:], t["wdt"][l])
        woutw = [wp.tile([P, DM], BF16, tag=f"wo{k}", name=f"wo{l}_{k}")
                 for k in range(ET)]
        for k in range(ET):
            nc.sync.dma_start(woutw[k][:], t["wout"][l, k * P:(k + 1) * P, :])
        amat = [wp.tile([P, S], F32, tag=f"am{m}", name=f"am{l}_{m}")
                for m in range(ET)]
        for m in range(ET):
            nc.sync.dma_start(amat[m][:], t["amat"][l, m * P:(m + 1) * P, :])
        pcol = [[wp.tile([P, 1], F32, tag=f"pc{w}_{m}", name=f"pc{w}{l}_{m}")
                 for m in range(ET)] for w in range(3)]
        for m in range(ET):
            sl = slice(m * P, (m + 1) * P)
            nc.sync.dma_start(pcol[0][m][:], t["bdt"][l, sl].unsqueeze(-1))
            nc.sync.dma_start(pcol[1][m][:], t["conv_b"][l, sl].unsqueeze(-1))
            nc.sync.dma_start(pcol[2][m][:], t["dvec"][l, sl].unsqueeze(-1))
        gcol = [wp.tile([P, 1], F32, tag=f"gc{i}", name=f"gc{l}_{i}")
                for i in range(KD)]
        bcol = [wp.tile([P, 1], F32, tag=f"bc{i}", name=f"bc{l}_{i}")
                for i in range(KD)]
        for i in range(KD):
            sl = slice(i * P, (i + 1) * P)
            nc.sync.dma_start(gcol[i][:], t["ln_g"][l, sl].unsqueeze(-1))
            nc.sync.dma_start(bcol[i][:], t["ln_b"][l, sl].unsqueeze(-1))
        st.update(winw=winw, convw=convw, wxw=wxw, wdtw=wdtw, woutw=woutw,
                  amat=amat, bdtc=pcol[0], cbc=pcol[1], dvc=pcol[2],
                  gcol=gcol, bcol=bcol, usb=[None] * ET, ch={})
        LS[l] = st

    def emit_A(l, c):
        """LN -> Win-u -> conv -> silu -> Wx -> AllReduce -> z -> dt -> vb
        for chunk c of layer l."""
        st = LS[l]
        cs = slice(c * Q, (c + 1) * Q)
        uid = f"{l}_{c}"
        # -- LN stats over the feature (partition) axis
        sxq = ps.tile([33, Q], F32, tag="st", name=f"sxq{uid}")
        sx, sq = sxq[0:1], sxq[32:33]
        for i in range(KD):
            xsq = tp.tile([P, Q], BF16, tag="xsqb", bufs=2, name=f"xsq{uid}_{i}")
            nc.scalar.square(xsq[:], x[i][:, cs])
            nc.tensor.matmul(sx[:], ones_col[:], x[i][:, cs],
                             start=(i == 0), stop=(i == KD - 1))
            nc.tensor.matmul(sq[:], ones_bf[:], xsq[:],
                             start=(i == 0), stop=(i == KD - 1))
        nm = ck.tile([1, Q], BF16, tag="nm", name=f"nm{uid}")
        rstd = ck.tile([1, Q], BF16, tag="rstd", name=f"rstd{uid}")
        nc.vector.tensor_scalar_mul(nm[0:1], sx[:], -1.0 / DM)
        nc.vector.tensor_scalar_mul(rstd[0:1], sq[:], 1.0 / DM)
        nc.vector.tensor_mul(sq[:], nm[0:1], nm[0:1])
        nc.vector.tensor_sub(rstd[0:1], rstd[0:1], sq[:])
        nc.vector.tensor_scalar_add(rstd[0:1], rstd[0:1], 1e-5)
        nc.scalar.activation(rstd[0:1], rstd[0:1], AF.Ln)
        nc.scalar.activation(rstd[0:1], rstd[0:1], AF.Exp, scale=-0.5)
        nmb = ck.tile([P, Q], BF16, tag="nmb", name=f"nmb{uid}")
        rsb = ck.tile([P, Q], BF16, tag="rsb", name=f"rsb{uid}")
        nc.gpsimd.partition_broadcast(nmb[:], nm[0:1])
        nc.gpsimd.partition_broadcast(rsb[:], rstd[0:1])
        hln = [ck.tile([P, Q], BF16, tag=f"hln{i}", name=f"hln{uid}_{i}")
               for i in range(KD)]
        for i in range(KD):
            t1 = tp.tile([P, Q], F32, tag="t4", bufs=2, name=f"lnt1_{uid}_{i}")
            nc.vector.tensor_add(t1[:], x[i][:, cs], nmb[:])
            nc.vector.tensor_mul(t1[:], t1[:], rsb[:])
            nc.vector.tensor_scalar(hln[i][:], t1[:], st["gcol"][i][:],
                                    st["bcol"][i][:], op0=OP.mult, op1=OP.add)
        # -- Win u-wave + conv + silu
        usi = [ck.tile([P, Q], BF16, tag=f"usi{m}", name=f"usi{uid}_{m}")
               for m in range(ET)]
        for m in range(ET):
            pu = pm.tile([P, Q], F32, tag="gemm", name=f"pu{uid}_{m}")
            for k in range(KD):
                nc.tensor.matmul(pu[:], st["winw"][k][:, m * P:(m + 1) * P],
                                 hln[k][:], start=(k == 0), stop=(k == KD - 1))
            if c == 0:
                st["usb"][m] = usb.tile([P, K - 1 + N], BF16, tag=f"usb{m}",
                                        name=f"usb{l}_{m}")
                nc.vector.memset(st["usb"][m][:, 0:K - 1], 0.0)
            u_sb = st["usb"][m]
            nc.scalar.copy(u_sb[:, K - 1 + c * Q:K - 1 + (c + 1) * Q], pu[:])
            pc = pm.tile([P, Q], F32, tag="gemm", name=f"pcv{uid}_{m}")
            for j in range(K):
                nc.tensor.matmul(pc[:], st["convw"][j][m][:],
                                 u_sb[:, c * Q + j:c * Q + j + Q],
                                 start=(j == 0), stop=(j == K - 1))
            apply_silu(usi[m][:], pc[:], st["cbc"][m][:], f"u{uid}_{m}")
        # -- Wx partial GEMM + pair AllReduce (bf16)
        px = pm.tile([P, Q], F32, tag="gemm", name=f"px{uid}")
        for k in range(ET):
            nc.tensor.matmul(px[0:RK + 2 * S, :], st["wxw"][k][:], usi[k][:],
                             start=(k == 0), stop=(k == ET - 1))
        xdp = dram.tile([RK + 2 * S, Q], BF16, tag="xdp", name=f"xdp{uid}")
        xds = dram.tile([RK + 2 * S, Q], BF16, tag="xds", name=f"xds{uid}")
        pxs = tp.tile([RK + 2 * S, Q], BF16, tag="pxsb", bufs=2,
                      name=f"pxs{uid}")
        nc.scalar.copy(pxs[:], px[0:RK + 2 * S, :])
        nc.sync.dma_start(xdp[:], pxs[:])
        nc.gpsimd.collective_compute(
            "AllReduce", OP.add, replica_groups=eh_groups,
            ins=[xdp[:]], outs=[xds[:]])
        # -- Win z-wave + silu (overlaps the collective)
        zsi = [ck.tile([P, Q], BF16, tag=f"zsi{m}", name=f"zsi{uid}_{m}")
               for m in range(ET)]
        for m in range(ET):
            pz = pm.tile([P, Q], F32, tag="gemm", name=f"pz{uid}_{m}")
            for k in range(KD):
                nc.tensor.matmul(pz[:],
                                 st["winw"][k][:, (ET + m) * P:(ET + m + 1) * P],
                                 hln[k][:], start=(k == 0), stop=(k == KD - 1))
            apply_silu(zsi[m][:], pz[:], None, f"z{uid}_{m}")
        st["ch"][c] = dict(usi=usi, zsi=zsi, xds=xds)

    def emit_A_dt(l, c):
        """dt = softplus(xdbl[:,:RK] @ Wdt + bdt).  Emitted well after the
        Wx AllReduce was issued, so the scalar queue is not convoyed behind
        the collective round trip."""
        st = LS[l]
        chs = st["ch"][c]
        uid = f"{l}_{c}"
        xdbl_bf = ck.tile([RK, Q], BF16, tag="xdblb", name=f"xdblb{uid}")
        nc.sync.dma_start(xdbl_bf[:], chs["xds"][0:RK, :])
        dtb = [ck.tile([P, Q], BF16, tag=f"dtb{m}", name=f"dtb{uid}_{m}")
               for m in range(ET)]
        spxs = []
        for m in range(ET):
            pd = pm.tile([P, Q], F32, tag="gemm", name=f"pd{uid}_{m}")
            nc.tensor.matmul(pd[:], st["wdtw"][:, m * P:(m + 1) * P],
                             xdbl_bf[:], start=True, stop=True)
            spx = tp.tile([P, Q], BF16, tag="spx", bufs=4, name=f"spx{uid}_{m}")
            nc.scalar.activation(spx[:], pd[:], AF.Exp, bias=st["bdtc"][m][:])
            spxs.append(spx)
        for m in range(ET):
            nc.scalar.activation(dtb[m][:], spxs[m][:], AF.Ln,
                                 bias=ones_col[:])
        chs["dtb"] = dtb

    def emit_B(l, c, embeds):
        """Scan units for chunk c of layer l; embeds[u] callables are
        emitted after unit u (to thread other chunks' work through the
        DVE-paced stream)."""
        st = LS[l]
        chs = st["ch"][c]
        vb = [ck.tile([P, Q], BF16, tag=f"vb{m}", name=f"vb{l}_{c}_{m}")
              for m in range(ET)]
        for m in range(ET):
            nc.vector.tensor_mul(vb[m][:], chs["dtb"][m][:],
                                 chs["usi"][m][:])
        chs["vb"] = vb
        yacc = [psY.tile([P, Q], F32, tag=f"yac{m}", name=f"yac{l}_{c}_{m}")
                for m in range(ET)]
        chs["yacc"] = yacc
        for g in range(NG):
            bb = scn.tile([P, GF], BF16, tag="bb", name=f"bb{l}_{c}_{g}")
            cc = scn.tile([P, GF], BF16, tag="cc", name=f"cc{l}_{c}_{g}")
            nc.sync.dma_start(
                bb[:],
                chs["xds"][RK + g * SG:RK + (g + 1) * SG,
                           :].partition_broadcast(P))
            nc.sync.dma_start(
                cc[:],
                chs["xds"][RK + S + g * SG:RK + S + (g + 1) * SG,
                           :].partition_broadcast(P))
            bb3 = bb.rearrange("p (s n) -> p s n", s=SG)
            for m in range(ET):
                u = g * ET + m
                uid = f"{l}_{c}_{g}_{m}"
                da = scn.tile([P, GF], BF16, tag="da", name=f"da{uid}")
                for si in range(SG):
                    s = g * SG + si
                    nc.scalar.activation(da[:, si * Q:(si + 1) * Q],
                                         chs["dtb"][m][:], AF.Exp,
                                         scale=st["amat"][m][:, s:s + 1])
                da3 = da.rearrange("p (s n) -> p s n", s=SG)
                dbu = scn.tile([P, GF], BF16, tag="dbu", bufs=2,
                               name=f"dbu{uid}")
                vv = chs["vb"][m].unsqueeze(1).broadcast_to((P, SG, Q))
                db3 = dbu.rearrange("p (s n) -> p s n", s=SG)
                nc.vector.tensor_mul(db3[:], vv, bb3[:])
                if c > 0:
                    # fold the cross-chunk carry into dbu[., si, 0]
                    for si in range(SG):
                        col = u * SG + si
                        nc.vector.scalar_tensor_tensor(
                            db3[:, si, 0:1], da3[:, si, 0:1],
                            carry[:, col:col + 1], db3[:, si, 0:1],
                            op0=OP.mult, op1=OP.add)
                nc.vector.memset(da3[:, :, 0:1], 0.0)
                hh = scn.tile([P, GF], BF16, tag="dbu", bufs=2,
                              name=f"hh{uid}")
                nc.vector.tensor_tensor_scan(hh[:], da[:], dbu[:], 0.0,
                                             op0=OP.mult, op1=OP.add)
                if c < CH - 1:
                    hh3 = hh.rearrange("p (s n) -> p s n", s=SG)
                    nc.vector.tensor_copy(
                        carry[:, u * SG:(u + 1) * SG], hh3[:, :, Q - 1])
                ym = scn.tile([P, GF], BF16, tag="ymt", bufs=2,
                              name=f"ym{uid}")
                nc.vector.tensor_mul(ym[:], hh[:], cc[:])
                for si in range(SG):
                    nc.tensor.matmul(yacc[m][:], identb[:],
                                     ym[:, si * Q:(si + 1) * Q],
                                     start=(g == 0 and si == 0),
                                     stop=(g == NG - 1 and si == SG - 1))
                if g == NG - 1:
                    # gate m as soon as its yacc closes; Wout k-major so
                    # its first matmuls overlap the remaining scan units
                    gated = chs.setdefault("gated", [None] * ET)
                    gated[m] = ck.tile([P, Q], BF16, tag=f"gt{m}",
                                       name=f"gt{l}_{c}_{m}")
                    nc.vector.scalar_tensor_tensor(
                        gated[m][:], chs["usi"][m][:], st["dvc"][m][:],
                        yacc[m][:], op0=OP.mult, op1=OP.add)
                    nc.vector.tensor_mul(gated[m][:], gated[m][:],
                                         chs["zsi"][m][:])
                for fn in embeds.get(u, ()):
                    fn()

    def emit_C(l, c):
        """Wout partial + pair AllReduce for chunk c (residual deferred)."""
        st = LS[l]
        chs = st["ch"][c]
        uid = f"{l}_{c}"
        dxp = dram.tile([DM, Q], BF16, tag="dxp", name=f"dxp{uid}")
        dxs = dram.tile([DM, Q], BF16, tag="dxs", name=f"dxs{uid}")
        chs["dxs"] = dxs
        po = [psY.tile([P, Q], F32, tag=f"yac{mo}", name=f"po{uid}_{mo}")
              for mo in range(KD)]
        for k in range(ET):
            for mo in range(KD):
                nc.tensor.matmul(po[mo][:],
                                 st["woutw"][k][:, mo * P:(mo + 1) * P],
                                 chs["gated"][k][:],
                                 start=(k == 0), stop=(k == ET - 1))
        for mo in range(KD):
            pos = tp.tile([P, Q], BF16, tag="bpd", bufs=2,
                          name=f"pos{uid}_{mo}")
            nc.scalar.copy(pos[:], po[mo][:])
            nc.sync.dma_start(dxp[mo * P:(mo + 1) * P, :], pos[:])
            if mo % 2 == 1:
                nc.gpsimd.collective_compute(
                    "AllReduce", OP.add, replica_groups=eh_groups,
                    ins=[dxp[(mo - 1) * P:(mo + 1) * P, :]],
                    outs=[dxs[(mo - 1) * P:(mo + 1) * P, :]])

    def emit_resid(l, c, mos):
        st = LS[l]
        chs = st["ch"][c]
        cs = slice(c * Q, (c + 1) * Q)
        for mo in mos:
            dxt = tp.tile([P, Q], BF16, tag="bpd", bufs=2,
                          name=f"dxt{l}_{c}_{mo}")
            nc.gpsimd.dma_start(dxt[:], chs["dxs"][mo * P:(mo + 1) * P, :])
            nc.vector.tensor_add(x[mo][:, cs], x[mo][:, cs], dxt[:])

    # ================= combine helpers =================
    DM2 = DM + 2
    cat_part = [dram.tile([DM2, Q], BF16, tag=f"catp{c}", bufs=1,
                          name=f"cat_part{c}") for c in range(CH)]
    cat_sum = [dram.tile([2 * DM2, Q], BF16, tag=f"cats{c}", bufs=1,
                         name=f"cat_sum{c}") for c in range(CH)]

    def emit_cat(c):
        """Ship our direction's chunk-c output (straight, no flip) plus its
        LN partial stats, then AllGather the dir pair."""
        cs = slice(c * Q, (c + 1) * Q)
        sxq = ps.tile([33, Q], F32, tag="st", name=f"csxq{c}")
        for i in range(KD):
            sf = tp.tile([P, Q], BF16, tag="bpd", bufs=2, name=f"sf{c}_{i}")
            nc.vector.tensor_copy(sf[:], x[i][:, cs])
            nc.sync.dma_start(cat_part[c][i * P:(i + 1) * P, :], sf[:])
            xsq = tp.tile([P, Q], BF16, tag="xsqb", bufs=2, name=f"pxq{c}_{i}")
            nc.scalar.square(xsq[:], sf[:])
            nc.tensor.matmul(sxq[0:1, :], ones_bf[:], sf[:],
                             start=(i == 0), stop=(i == KD - 1))
            nc.tensor.matmul(sxq[32:33, :], ones_bf[:], xsq[:],
                             start=(i == 0), stop=(i == KD - 1))
        stats_sb = tp.tile([33, Q], BF16, tag="cstat", bufs=2,
                           name=f"stats_sb{c}")
        nc.scalar.copy(stats_sb[0:1, :], sxq[0:1, :])
        nc.scalar.copy(stats_sb[32:33, :], sxq[32:33, :])
        nc.sync.dma_start(cat_part[c][DM:DM + 1, :], stats_sb[0:1, :])
        nc.sync.dma_start(cat_part[c][DM + 1:DM + 2, :], stats_sb[32:33, :])
        nc.gpsimd.collective_compute(
            "AllGather", OP.bypass, replica_groups=dir_groups,
            ins=[cat_part[c][:]], outs=[cat_sum[c][:]])

    # ================= emission schedule =================
    load_weights(0)
    # combine weights prefetch (idle DMA time at the start)
    cww = [wp.tile([P, DM], BF16, tag=f"cwt{k}", name=f"cw{k}")
           for k in range(2 * KD)]
    for k in range(2 * KD):
        nc.sync.dma_start(cww[k][:], t["cw"][k * P:(k + 1) * P, :])
    cbias_c = [wp.tile([P, 1], F32, tag=f"cbs{m}", name=f"cbs{m}")
               for m in range(KD)]
    for m in range(KD):
        nc.sync.dma_start(cbias_c[m][:],
                          t["cbias"][m * P:(m + 1) * P].unsqueeze(-1))
    cgcol = [wp.tile([P, 1], F32, tag="cgcol", bufs=8, name=f"cgc{i}")
             for i in range(2 * KD)]
    cbcol = [wp.tile([P, 1], F32, tag="cbcol", bufs=8, name=f"cbc{i}")
             for i in range(2 * KD)]
    for i in range(2 * KD):
        nc.sync.dma_start(cgcol[i][:], t["cg"][i * P:(i + 1) * P].unsqueeze(-1))
        nc.sync.dma_start(cbcol[i][:], t["cb"][i * P:(i + 1) * P].unsqueeze(-1))
    emit_A(0, 0)
    emit_A_dt(0, 0)
    for l in range(L):
        embeds0 = {}
        if l > 0:
            embeds0[2] = [lambda l=l: emit_resid(l - 1, 1, (0, 1))]
            embeds0[3] = [lambda l=l: emit_resid(l - 1, 1, (2, 3))]
        embeds0[4] = [lambda l=l: emit_A(l, 1)]
        embeds0[10] = [lambda l=l: emit_A_dt(l, 1)]
        emit_B(l, 0, embeds0)
        emit_C(l, 0)
        embeds1 = {
            2: [lambda l=l: emit_resid(l, 0, (0, 1))],
            3: [lambda l=l: emit_resid(l, 0, (2, 3))],
        }
        if l < L - 1:
            embeds1[4] = [lambda l=l: (load_weights(l + 1),
                                       emit_A(l + 1, 0))]
            embeds1[10] = [lambda l=l: emit_A_dt(l + 1, 0)]
        else:
            embeds1[9] = [lambda: emit_cat(0)]
        emit_B(l, 1, embeds1)
        emit_C(l, 1)
    emit_resid(L - 1, 1, (0, 1, 2, 3))
    emit_cat(1)

    # ================= combine =================
    def cat_row(i, c):
        """Feature-tile i of chunk c of the concat layout."""
        if i < KD:
            return cat_sum[c][i * P:(i + 1) * P, :]
        return cat_sum[c][DM2 + (i - KD) * P:DM2 + (i - KD + 1) * P, :]

    # global LN stats in OUTPUT token order: fwd stats straight + bwd
    # stats column-reversed (bwd stream position p holds token N-1-p).
    cnm = ck.tile([1, N], BF16, tag="cnm", name="cnm")
    crstd = ck.tile([1, N], BF16, tag="crstd", name="crstd")
    sxb = ck.tile([1, N], BF16, tag="sxb", name="sxb")
    sqb = ck.tile([1, N], BF16, tag="sqb", name="sqb")
    for c in range(CH):
        cs = slice(c * Q, (c + 1) * Q)
        nc.sync.dma_start(cnm[0:1, cs], cat_sum[c][DM:DM + 1, :])
        nc.sync.dma_start(crstd[0:1, cs], cat_sum[c][DM + 1:DM + 2, :])
        nc.sync.dma_start(sxb[0:1, cs], cat_sum[c][DM2 + DM:DM2 + DM + 1, :])
        nc.sync.dma_start(sqb[0:1, cs],
                          cat_sum[c][DM2 + DM + 1:DM2 + DM + 2, :])
    nc.vector.tensor_add(cnm[0:1, :], cnm[0:1, :], sxb[0:1, ::-1])
    nc.vector.tensor_add(crstd[0:1, :], crstd[0:1, :], sqb[0:1, ::-1])
    nc.vector.tensor_scalar_mul(cnm[0:1, :], cnm[0:1, :], -1.0 / (2 * DM))
    nc.vector.tensor_scalar_mul(crstd[0:1, :], crstd[0:1, :], 1.0 / (2 * DM))
    nc.vector.tensor_mul(sxb[0:1, :], cnm[0:1, :], cnm[0:1, :])
    nc.vector.tensor_sub(crstd[0:1, :], crstd[0:1, :], sxb[0:1, :])
    nc.vector.tensor_scalar_add(crstd[0:1, :], crstd[0:1, :], 1e-5)
    nc.scalar.activation(crstd[0:1, :], crstd[0:1, :], AF.Ln)
    nc.scalar.activation(crstd[0:1, :], crstd[0:1, :], AF.Exp, scale=-0.5)
    cnmb = ck.tile([P, N], BF16, tag="cnmb", bufs=1, name="cnmb")
    crsb = ck.tile([P, N], BF16, tag="crsb", bufs=1, name="crsb")
    nc.gpsimd.partition_broadcast(cnmb[:], cnm[0:1, :])
    nc.gpsimd.partition_broadcast(crsb[:], crstd[0:1, :])

    for q in range(N // CQ):
        qs = slice(q * CQ, (q + 1) * CQ)
        rq = N - (q + 1) * CQ      # bwd source cols (to be read reversed)
        xc = [ck.tile([P, CQ], BF16, tag="xc", bufs=9, name=f"xc{q}_{i}")
              for i in range(2 * KD)]
        for i in range(2 * KD):
            if i < KD:
                c0, o0 = divmod(q * CQ, Q)
                src = cat_row(i, c0)[:, o0:o0 + CQ]
            else:
                c0, o0 = divmod(rq, Q)
                src = cat_row(i, c0)[:, o0:o0 + CQ]
            nc.sync.dma_start(xc[i][:], src)
        hcq = [ck.tile([P, CQ], BF16, tag="hc", bufs=9, name=f"hc{q}_{i}")
               for i in range(2 * KD)]
        for i in range(2 * KD):
            xin = xc[i][:, :] if i < KD else xc[i][:, ::-1]
            t1c = tp.tile([P, CQ], F32, tag="lnt1c", bufs=2, name=f"t1c{q}_{i}")
            nc.vector.tensor_add(t1c[:], xin, cnmb[:, qs])
            nc.vector.tensor_mul(t1c[:], t1c[:], crsb[:, qs])
            nc.vector.tensor_scalar(hcq[i][:], t1c[:], cgcol[i][:],
                                    cbcol[i][:], op0=OP.mult, op1=OP.add)
        ot = tp.tile([P, DM], F32, tag="tio", bufs=2, name=f"ot{q}_a")
        ot2 = tp.tile([P, DM], F32, tag="tio", bufs=2, name=f"ot{q}_b")
        for m in range(KD):
            pg = pm.tile([P, Q], F32, tag="gemm", name=f"pg{q}_{m}")
            for k in range(2 * KD):
                nc.tensor.matmul(pg[:, 0:CQ], cww[k][:, m * P:(m + 1) * P],
                                 hcq[k][:], start=(k == 0),
                                 stop=(k == 2 * KD - 1))
            ogm = tp.tile([P, CQ], F32, tag="og", bufs=2, name=f"og{q}_{m}")
            gfn = AF.Identity if sim_compat else AF.Gelu
            nc.scalar.activation(ogm[:, 0:CQ], pg[:, 0:CQ], gfn,
                                 bias=cbias_c[m][:])
            for hh2 in range(CQ // P):
                pts = pm.tile([P, Q], F32, tag="gemm", name=f"otp{q}_{m}_{hh2}")
                nc.tensor.transpose(
                    pts[:, 0:P], ogm[:, hh2 * P:(hh2 + 1) * P], ident[:])
                dst = ot if hh2 == 0 else ot2
                nc.scalar.copy(dst[:, m * P:(m + 1) * P], pts[:, 0:P])
        nc.sync.dma_start(out_ap[q * CQ:q * CQ + P, :], ot[:])
        nc.sync.dma_start(out_ap[q * CQ + P:(q + 1) * CQ, :], ot2[:])

    es.close()


# ----------------------------------------------------------------- host side
def _bf(a):
    return np.asarray(a, dtype=np.float32).astype(ml_dtypes.bfloat16)


def _core_inputs(inputs, b, dirn, e):
    pre = "fwd" if dirn == 0 else "bwd"
    g = lambda n: np.asarray(inputs[pre + "_" + n], dtype=np.float32)
    x = np.asarray(inputs["x"], dtype=np.float32)[b]          # (N, DM)
    if dirn == 1:
        x = x[::-1]
    es = slice(e * EL, (e + 1) * EL)

    win_full = g("Win")                                        # (L, DM, 2*EFULL)
    win = np.concatenate(
        [win_full[:, :, e * EL:(e + 1) * EL],
         win_full[:, :, EFULL + e * EL:EFULL + (e + 1) * EL]], axis=2)

    cw4 = g("conv_w")[:, es, 0, :]                             # (L, EL, K)
    convd = np.zeros((L, K, ET, P, P), np.float32)
    for j in range(K):
        for m in range(ET):
            for l in range(L):
                np.fill_diagonal(convd[l, j, m], cw4[l, m * P:(m + 1) * P, j])

    return {
        "x_in": np.ascontiguousarray(x.T),
        "ident": np.eye(P, dtype=np.float32),
        "ones_col": np.ones((P, 1), np.float32),
        "ln_g": g("ln_g"), "ln_b": g("ln_b"),
        "win": _bf(win),
        "convd": _bf(convd),
        "conv_b": g("conv_b")[:, es],
        "wx": _bf(g("Wx")[:, es, :]),
        "wdt": _bf(g("Wdt")[:, :, es]),
        "bdt": g("bdt")[:, es],
        "amat": -np.exp(g("Alog")[:, es, :]),
        "dvec": g("D")[:, es],
        "wout": _bf(g("Wout")[:, es, :]),
        "cg": np.asarray(inputs["cmb_ln_g"], np.float32),
        "cb": np.asarray(inputs["cmb_ln_b"], np.float32),
        "cw": _bf(np.asarray(inputs["cmb_W"], np.float32)),
        "cbias": np.asarray(inputs["cmb_b"], np.float32),
    }


def make_in_maps(inputs):
    in_maps = []
    for b in range(B):
        for dirn in range(2):
            for e in range(2):
                in_maps.append(_core_inputs(inputs, b, dirn, e))
    return in_maps


def kernel(**inputs):
    nc = _build_program()
    res = bass_utils.run_bass_kernel_spmd(nc, make_in_maps(inputs),
                                          list(range(8)))
    out = np.empty((B, N, DM), np.float32)
    for b in range(B):
        out[b] = res.results[b * 4]["out"]
    return out


if __name__ == "__main__":
    nc = _build_program()
    n_inst = sum(len(bb.instructions) for f in nc.m.functions for bb in f.blocks)
    print("program built ok:", n_inst, "instructions")
